# revision 1
# baseline (speedup 1.0000x reference)
# Mistral sliding-window attention (B=1, S=2048, H=4096, 32 q heads / 8 kv
# heads, window 4096 -> plain causal at this S) on 8 Trainium2 NeuronCores.
#
# Sharding: tensor-parallel over heads. Core c owns q heads 4c..4c+3 and kv
# head c. hidden_states is replicated (transposed on host to [H, S] so the
# contraction dim is the partition dim). Each core computes its attention
# output slice attn.T [512, S]; per-head AllGathers assemble the full
# [4096, S] while later heads still compute, and each core accumulates a
# 512-column slice of o_proj head-by-head; the host concatenates the 8
# column slices into the full output.
#
# All big matmuls run as float32r (fp32 storage, full-rate PE) with the
# moving dim = 512. Scores are computed transposed (S.T[kv, q]) so that the
# P@V contraction needs no transposes of the probability tiles; softmax
# denominators come from an all-ones stationary matmul accumulated alongside
# P@V, and the causal mask is a host-precomputed staircase slice multiplied
# in after exp. Attention runs two-pass per (head, q-chunk) — all score
# matmuls + exps first, then the PV/sum matmuls — so the PE never stalls on
# the ACT engine mid-chain.

from contextlib import ExitStack

import numpy as np

import concourse.bacc as bacc
import concourse.bass as bass
import concourse.mybir as mybir
import concourse.tile as tile
from concourse.bass_utils import run_bass_kernel_spmd
from concourse.masks import make_identity

HIDDEN = 4096
NH = 32
NKV = 8
HD = 128
THETA = 10000.0
S = 2048
NCORES = 8

QH = NH // NCORES          # 4 q heads per core
DQ = QH * HD               # 512 (per-core q/attn width)
DOUT = DQ + 2 * HD         # 768 = q heads + k + v projection width
MT = DOUT // 128           # 6 projection m-tiles (0..3 q, 4 k, 5 v)
KT = HIDDEN // 128         # 32 contraction tiles
KG = 4                     # x-load group: k-tiles per DMA
TCH = 512                  # token chunk (matmul moving dim)
NTCH = S // TCH            # 4
KVT = S // 128             # 16 kv tiles
SCALE = 1.0 / float(np.sqrt(HD))

F32 = mybir.dt.float32
F32R = mybir.dt.float32r
EXP = mybir.ActivationFunctionType.Exp


def _rope(nc, pool, src, dst, cs, sn):
    """dst = src*cos + rotate_half(src)*sin, in [d, tok] layout.

    src/dst are [128, n]; cs/sn are [64, n] (the two 64-row halves share
    frequencies). rotate_half: rows 0:64 get -src[64:128], rows 64:128 get
    src[0:64].
    """
    top, bot = src[0:64, :], src[64:128, :]
    ta = pool.tile([64, TCH], F32, name="rope_a")
    tb = pool.tile([64, TCH], F32, name="rope_b")
    nc.vector.tensor_mul(ta, top, cs)
    nc.vector.tensor_mul(tb, bot, sn)
    nc.vector.tensor_sub(dst[0:64, :], ta, tb)
    nc.vector.tensor_mul(ta, bot, cs)
    nc.vector.tensor_mul(tb, top, sn)
    nc.vector.tensor_add(dst[64:128, :], ta, tb)


def build_kernel_body(ctx: ExitStack, tc: tile.TileContext, outs, ins):
    nc = tc.nc
    xT, wqkv, ow, cos_t, sin_t, stair = (
        ins["xT"], ins["wqkv"], ins["ow"], ins["cos_t"], ins["sin_t"], ins["stair"],
    )
    out = outs["out"]

    # per-head bounce + gather buffers so each head's AllGather can fire as
    # soon as that head's attention is done (overlaps comm with compute)
    attn_loc = [nc.dram_tensor(f"attn_loc{h}", [HD, S], F32).ap()
                for h in range(QH)]
    attn_gat = [nc.dram_tensor(f"attn_gat{h}", [NCORES * HD, S], F32,
                               addr_space="Shared").ap()
                for h in range(QH)]

    singles = ctx.enter_context(tc.tile_pool(name="singles", bufs=1))
    stair_sb = singles.tile([128, 896], F32)
    nc.sync.dma_start(out=stair_sb, in_=stair)
    ones_sb = singles.tile([128, 128], F32R)

    # persistent projection outputs, [d, tok] layout
    qT = singles.tile([128, QH, S], F32R)    # q head h -> qT[:, h, :]
    kT = singles.tile([128, S], F32R)
    V = singles.tile([128, KVT, HD], F32R)   # V[:, j, :] = [tok 128, d 128]

    # ---- phase 1: QKV projection + RoPE --------------------------------
    with (
        tc.tile_pool(name="wq", bufs=1) as wp,
        tc.tile_pool(name="xt", bufs=3) as xp,
        tc.tile_pool(name="rope", bufs=2) as rp,
        tc.tile_pool(name="p1ps", bufs=1, space="PSUM") as pp1,
    ):
        cos_sb = wp.tile([64, S], F32)
        sin_sb = wp.tile([64, S], F32)
        vT = wp.tile([128, S], F32)
        ident_sb = wp.tile([128, 128], F32)
        ones_f = wp.tile([128, 128], F32)
        nc.vector.memset(ones_f, 1.0)
        nc.vector.tensor_copy(ones_sb, ones_f)
        make_identity(nc, ident_sb)

        wq3 = wqkv.rearrange("(k p) d -> p k d", p=128)
        x3 = xT.rearrange("(k p) s -> p k s", p=128)
        # x chunk (t=0, kg=0) first so the first matmul starts almost
        # immediately; weight k-tiles follow in per-tile DMAs
        w_sb = [wp.tile([128, DOUT], F32R, name=f"w{k}", tag=f"w{k}")
                for k in range(KT)]
        xg0 = xp.tile([128, KG, TCH], F32R, name="xg")
        nc.sync.dma_start(out=xg0, in_=x3[:, 0:KG, 0:TCH])
        for k in range(KT):
            nc.sync.dma_start(out=w_sb[k], in_=wq3[:, k, :])
        nc.sync.dma_start(out=cos_sb, in_=cos_t)
        nc.sync.dma_start(out=sin_sb, in_=sin_t)
        for t in range(NTCH):
            ps = [pp1.tile([128, TCH], F32, name=f"p1_{m}", tag=f"p1_{m}")
                  for m in range(MT)]
            for kg in range(KT // KG):
                if t == 0 and kg == 0:
                    xg = xg0
                else:
                    xg = xp.tile([128, KG, TCH], F32R, name="xg")
                    nc.sync.dma_start(
                        out=xg,
                        in_=x3[:, kg * KG:(kg + 1) * KG, t * TCH:(t + 1) * TCH])
                for ki in range(KG):
                    k = kg * KG + ki
                    for m in range(MT):
                        nc.tensor.matmul(
                            ps[m],
                            lhsT=w_sb[k][:, m * 128:(m + 1) * 128],
                            rhs=xg[:, ki, :],
                            start=(k == 0), stop=(k == KT - 1),
                        )
            cs = cos_sb[:, t * TCH:(t + 1) * TCH]
            sn = sin_sb[:, t * TCH:(t + 1) * TCH]
            for h in range(QH):
                _rope(nc, rp, ps[h], qT[:, h, t * TCH:(t + 1) * TCH], cs, sn)
            _rope(nc, rp, ps[QH], kT[:, t * TCH:(t + 1) * TCH], cs, sn)
            nc.scalar.copy(out=vT[:, t * TCH:(t + 1) * TCH], in_=ps[QH + 1])
            # V = vT.T for this chunk's kv tiles (PE transpose [d,tok]->[tok,d])
            for j in range(4 * t, 4 * t + 4):
                pv = pp1.tile([128, 128], F32, name="pvt", tag="pvt")
                nc.tensor.transpose(pv, vT[:, j * 128:(j + 1) * 128], ident_sb)
                nc.scalar.copy(out=V[:, j, :], in_=pv)

    # ---- phases 2+3: attention heads with per-head AllGather; o_proj for
    # head 0 interleaved into attention head 3, rest at the tail ---------
    with (
        tc.tile_pool(name="pt", bufs=8) as ptp,
        tc.tile_pool(name="ao", bufs=2) as aop,
        tc.tile_pool(name="ow", bufs=16) as owp,
        tc.tile_pool(name="at", bufs=2) as atp,
        tc.tile_pool(name="acc", bufs=1) as accp,
        tc.tile_pool(name="p2sc", bufs=2, space="PSUM") as pp2,
        tc.tile_pool(name="p2acc", bufs=1, space="PSUM") as pa2,
        tc.tile_pool(name="p3ps", bufs=1, space="PSUM") as pp3,
    ):
        # o_proj output accumulator: acc[:, b, :] = out rows b*128:(b+1)*128
        acc = accp.tile([128, S // 128, TCH], F32)
        ow3 = ow.rearrange("(k p) d -> p k d", p=128)
        ag3 = [attn_gat[h].rearrange("(r p) s -> p r s", p=128)
               for h in range(QH)]

        def attention_chunk(h, c):
            """One (head, q-chunk): scores+exp in j-pair waves, then PV."""
            jmax = 4 * c + 3
            po = pa2.tile([128, TCH], F32, name="po", tag="po")
            psum_s = pa2.tile([128, TCH], F32, name="ps", tag="ps")
            qslice = qT[:, h, c * TCH:(c + 1) * TCH]
            npair = (jmax + 1) // 2
            PW = 4  # j-pairs per pass-A/B wave (bounds live pt tiles)
            for p0 in range(0, npair, PW):
                p1 = min(p0 + PW, npair)
                pts = []
                for p in range(p0, p1):
                    # two score matmuls into one 2-bank psum tile, one exp
                    sc = pp2.tile([128, 2, TCH], F32, name="sc", tag="sc")
                    for i in range(2):
                        j = 2 * p + i
                        nc.tensor.matmul(
                            sc[:, i, :], lhsT=kT[:, j * 128:(j + 1) * 128],
                            rhs=qslice, start=True, stop=True)
                    pt = ptp.tile([128, 2, TCH], F32R, name="pt", tag="pt")
                    nc.scalar.activation(pt, sc, EXP, scale=SCALE)
                    for i in range(2):
                        j = 2 * p + i
                        rdiag = j - 4 * c
                        if rdiag >= 0:  # tile touches the causal diagonal
                            off = 384 - rdiag * 128
                            nc.vector.tensor_mul(
                                pt[:, i, :], pt[:, i, :],
                                stair_sb[:, off:off + TCH])
                    pts.append(pt)
                for idx, p in enumerate(range(p0, p1)):
                    for i in range(2):
                        j = 2 * p + i
                        nc.tensor.matmul(po, lhsT=V[:, j, :],
                                         rhs=pts[idx][:, i, :],
                                         start=(j == 0), stop=(j == jmax))
                        nc.tensor.matmul(psum_s, lhsT=ones_sb,
                                         rhs=pts[idx][:, i, :],
                                         start=(j == 0), stop=(j == jmax))
            rec = aop.tile([128, TCH], F32, name="rec")
            nc.vector.reciprocal(rec, psum_s)
            ao = aop.tile([128, TCH], F32, name="ao")
            nc.vector.tensor_mul(ao, po, rec)
            nc.sync.dma_start(
                out=attn_loc[h][:, c * TCH:(c + 1) * TCH], in_=ao)

        def allgather_head(h):
            nc.gpsimd.collective_compute(
                "AllGather",
                mybir.AluOpType.bypass,
                ins=[attn_loc[h][:, :]],
                outs=[attn_gat[h][:, :]],
                replica_groups=[list(range(NCORES))],
            )

        def oproj_load_weights(h):
            ows = []
            for r in range(NCORES):
                owk = owp.tile([128, DQ], F32R, name="owk", tag="owk")
                nc.sync.dma_start(out=owk, in_=ow3[:, r * QH + h, :])
                ows.append(owk)
            return ows

        def oproj_chunk(h, g, ows):
            """acc[:, 4g:4g+4, :] += sum_r at(r, h) @ ow(r, h) for 512 toks."""
            at = atp.tile([128, NCORES, TCH], F32R, name="at", tag="at")
            nc.sync.dma_start(
                out=at, in_=ag3[h][:, :, g * TCH:(g + 1) * TCH].bitcast(F32R))
            for mp in range(2):
                pcs = [pp3.tile([128, TCH], F32, name=f"pc{i}", tag=f"pc{i}")
                       for i in range(2)]
                for r in range(NCORES):
                    for i, mi in enumerate((2 * mp, 2 * mp + 1)):
                        nc.tensor.matmul(
                            pcs[i],
                            lhsT=at[:, r, mi * 128:(mi + 1) * 128],
                            rhs=ows[r],
                            start=(r == 0), stop=(r == NCORES - 1),
                        )
                for i, mi in enumerate((2 * mp, 2 * mp + 1)):
                    b = g * 4 + mi
                    if h == 0:
                        nc.scalar.copy(out=acc[:, b, :], in_=pcs[i])
                    else:
                        nc.vector.tensor_add(acc[:, b, :], acc[:, b, :],
                                             pcs[i])

        for h in range(3):
            for c in range(NTCH):
                attention_chunk(h, c)
            allgather_head(h)
        # head 3 attention interleaved with o_proj of the gathered head 0
        ows0 = oproj_load_weights(0)
        for c in range(NTCH):
            attention_chunk(3, c)
            oproj_chunk(0, c, ows0)
        allgather_head(3)
        for h in range(1, QH):
            ows = oproj_load_weights(h)
            for g in range(S // TCH):
                oproj_chunk(h, g, ows)

        nc.sync.dma_start(out=out.rearrange("(b p) d -> p b d", p=128), in_=acc)


_NC_CACHE = None


def build_program():
    global _NC_CACHE
    if _NC_CACHE is not None:
        return _NC_CACHE
    nc = bacc.Bacc("TRN2", target_bir_lowering=False, debug=False,
                   num_devices=NCORES)
    ins = {
        "xT": nc.dram_tensor("xT", [HIDDEN, S], F32R, kind="ExternalInput").ap(),
        "wqkv": nc.dram_tensor("wqkv", [HIDDEN, DOUT], F32R,
                               kind="ExternalInput").ap(),
        "ow": nc.dram_tensor("ow", [HIDDEN, DQ], F32R, kind="ExternalInput").ap(),
        "cos_t": nc.dram_tensor("cos_t", [64, S], F32, kind="ExternalInput").ap(),
        "sin_t": nc.dram_tensor("sin_t", [64, S], F32, kind="ExternalInput").ap(),
        "stair": nc.dram_tensor("stair", [128, 896], F32,
                                kind="ExternalInput").ap(),
    }
    outs = {"out": nc.dram_tensor("out", [S, DQ], F32, kind="ExternalOutput").ap()}
    with tile.TileContext(nc) as tc:
        with ExitStack() as ctx:
            build_kernel_body(ctx, tc, outs, ins)
    nc.compile()
    _NC_CACHE = nc
    return nc


def make_in_maps(hidden_states, position_ids, q_w, k_w, v_w, o_w):
    x = np.asarray(hidden_states, dtype=np.float32).reshape(S, HIDDEN)
    xT = np.ascontiguousarray(x.T)
    pos = np.asarray(position_ids).reshape(S).astype(np.float64)
    inv = 1.0 / (THETA ** (np.arange(0, HD, 2, dtype=np.float64) / HD))
    fr = inv[:, None] * pos[None, :]                       # [64, S]
    cos_t = np.cos(fr).astype(np.float32)
    sin_t = np.sin(fr).astype(np.float32)
    u = np.arange(896, dtype=np.int64)[None, :]
    kvi = np.arange(128, dtype=np.int64)[:, None]
    stair = ((u - kvi) >= 384).astype(np.float32)          # [128, 896]

    q_w = np.asarray(q_w, dtype=np.float32)
    k_w = np.asarray(k_w, dtype=np.float32)
    v_w = np.asarray(v_w, dtype=np.float32)
    o_w = np.asarray(o_w, dtype=np.float32)

    in_maps = []
    for c in range(NCORES):
        wqkv = np.ascontiguousarray(np.concatenate(
            [q_w[:, c * DQ:(c + 1) * DQ],
             k_w[:, c * HD:(c + 1) * HD],
             v_w[:, c * HD:(c + 1) * HD]], axis=1))
        owc = np.ascontiguousarray(o_w[:, c * DQ:(c + 1) * DQ])
        in_maps.append({"xT": xT, "wqkv": wqkv, "ow": owc,
                        "cos_t": cos_t, "sin_t": sin_t, "stair": stair})
    return in_maps


def run(inputs: dict, trace: bool = False):
    """Run on the 8 NeuronCores; returns (full_output, BassKernelResults)."""
    nc = build_program()
    in_maps = make_in_maps(**inputs)
    res = run_bass_kernel_spmd(nc, in_maps, core_ids=list(range(NCORES)),
                               trace=trace)
    full = np.concatenate([res.results[c]["out"] for c in range(NCORES)], axis=1)
    return full.reshape(1, S, HIDDEN), res


def kernel(**inputs) -> np.ndarray:
    out, _ = run(inputs)
    return out



# revision 11
# speedup vs baseline: 1.0906x; 1.0906x over previous
# Mistral sliding-window attention (B=1, S=2048, H=4096, 32 q heads / 8 kv
# heads, window 4096 -> plain causal at this S) on 8 Trainium2 NeuronCores.
#
# Sharding: tensor-parallel over heads. Core c owns q heads 4c..4c+3 and kv
# head c. hidden_states is replicated (transposed on host to [H, S] so the
# contraction dim is the partition dim). Each core computes its attention
# output slice, per-head AllGathers (bf16) assemble the full attention while
# later heads still compute, and each core computes a 512-column slice of
# o_proj; the host concatenates the 8 column slices.
#
# v2 notes:
# - QKV path stays fp32 (float32r matmuls run full-rate at moving dim 512);
#   the o_proj path (attention outs -> AllGather -> gather reads -> o_w) is
#   bf16, halving collective+DMA bytes there at ~0.4% quantization error.
# - Causal mask is a 0/1 staircase multiplied into the probabilities
#   post-exp on the otherwise-idle gpsimd engine (psum bias pre-fill would
#   avoid the dependency but matmul start=False onto engine-written psum is
#   rejected by the accumulation-group model).
# - Attention is software-pipelined with one j-pair of lookahead: the PE
#   issues QK(p+1) before PV(p), hiding the ACT-engine exp latency.
# - attn_loc writes go out on the scalar engine's DMA queue; the bulky
#   loads (weights, x, gathered attention) stay on the sync queue. The
#   per-head AllGather trigger waits only on the scalar queue's few ao
#   writes, so it fires as soon as that head is done (the v1 kernel's
#   gathers all serialized after the whole attention phase because the
#   gather-input writes sat behind stalled gather-output reads in one
#   in-order queue).

from contextlib import ExitStack

import numpy as np

import concourse.bacc as bacc
import concourse.bass as bass
import concourse.mybir as mybir
import concourse.tile as tile
from concourse.bass_utils import run_bass_kernel_spmd
from concourse.masks import make_identity

HIDDEN = 4096
NH = 32
NKV = 8
HD = 128
THETA = 10000.0
S = 2048
NCORES = 8

QH = NH // NCORES          # 4 q heads per core
DQ = QH * HD               # 512 (per-core q/attn width)
DOUT = DQ + 2 * HD         # 768 = q heads + k + v projection width
MT = DOUT // 128           # 6 projection m-tiles (0..3 q, 4 k, 5 v)
KT = HIDDEN // 128         # 32 contraction tiles
KG = 4                     # x-load group: k-tiles per DMA
TCH = 512                  # token chunk (matmul moving dim)
NTCH = S // TCH            # 4
KVT = S // 128             # 16 kv tiles
SCALE = 1.0 / float(np.sqrt(HD))


F32 = mybir.dt.float32
F32R = mybir.dt.float32r
BF16 = mybir.dt.bfloat16
EXP = mybir.ActivationFunctionType.Exp


def _rope(nc, pool, src, dst, cs, sn):
    """dst = src*cos + rotate_half(src)*sin, in [d, tok] layout.

    src/dst are [128, n]; cs/sn are [64, n] (the two 64-row halves share
    frequencies). rotate_half: rows 0:64 get -src[64:128], rows 64:128 get
    src[0:64].
    """
    top, bot = src[0:64, :], src[64:128, :]
    ta = pool.tile([64, TCH], F32, name="rope_a")
    tb = pool.tile([64, TCH], F32, name="rope_b")
    nc.vector.tensor_mul(ta, top, cs)
    nc.vector.tensor_mul(tb, bot, sn)
    nc.vector.tensor_sub(dst[0:64, :], ta, tb)
    nc.vector.tensor_mul(ta, bot, cs)
    nc.vector.tensor_mul(tb, top, sn)
    nc.vector.tensor_add(dst[64:128, :], ta, tb)


def build_kernel_body(ctx: ExitStack, tc: tile.TileContext, outs, ins):
    nc = tc.nc
    xT, wqkv, ow, cos_t, sin_t, stair = (
        ins["xT"], ins["wqkv"], ins["ow"], ins["cos_t"], ins["sin_t"],
        ins["stair"],
    )
    out = outs["out"]

    # per-head bounce + gather buffers so each head's AllGather can fire as
    # soon as that head's attention is done (overlaps comm with compute)
    attn_loc = [nc.dram_tensor(f"attn_loc{h}", [HD, S], BF16).ap()
                for h in range(QH)]
    attn_gat = [nc.dram_tensor(f"attn_gat{h}", [NCORES * HD, S], BF16,
                               addr_space="Shared").ap()
                for h in range(QH)]

    singles = ctx.enter_context(tc.tile_pool(name="singles", bufs=1))
    stair_sb = singles.tile([128, 896], F32)
    nc.sync.dma_start(out=stair_sb, in_=stair)
    ones_sb = singles.tile([128, 128], F32R)

    # persistent projection outputs, [d, tok] layout
    qT = singles.tile([128, QH, S], F32R)    # q head h -> qT[:, h, :]
    kT = singles.tile([128, S], F32R)
    V = singles.tile([128, KVT, HD], F32R)   # V[:, j, :] = [tok 128, d 128]

    # ---- phase 1: QKV projection + RoPE --------------------------------
    with (
        tc.tile_pool(name="wq", bufs=1) as wp,
        tc.tile_pool(name="xt", bufs=3) as xp,
        tc.tile_pool(name="rope", bufs=2) as rp,
        tc.tile_pool(name="p1ps", bufs=1, space="PSUM") as pp1,
    ):
        cos_sb = wp.tile([64, S], F32)
        sin_sb = wp.tile([64, S], F32)
        vT = wp.tile([128, S], F32)
        ident_sb = wp.tile([128, 128], F32)
        ones_f = wp.tile([128, 128], F32)
        nc.vector.memset(ones_f, 1.0)
        nc.vector.tensor_copy(ones_sb, ones_f)
        make_identity(nc, ident_sb)

        wq3 = wqkv.rearrange("(k p) d -> p k d", p=128)
        x3 = xT.rearrange("(k p) s -> p k s", p=128)
        # x chunk (t=0, kg=0) first so the first matmul starts almost
        # immediately; weight k-tiles follow in per-tile DMAs
        w_sb = [wp.tile([128, DOUT], F32R, name=f"w{k}", tag=f"w{k}")
                for k in range(KT)]
        xg0 = xp.tile([128, KG, TCH], F32R, name="xg")
        nc.sync.dma_start(out=xg0, in_=x3[:, 0:KG, 0:TCH])
        for k in range(KT):
            nc.sync.dma_start(out=w_sb[k], in_=wq3[:, k, :])
        nc.sync.dma_start(out=cos_sb, in_=cos_t)
        nc.sync.dma_start(out=sin_sb, in_=sin_t)
        for t in range(NTCH):
            ps = [pp1.tile([128, TCH], F32, name=f"p1_{m}", tag=f"p1_{m}")
                  for m in range(MT)]
            for kg in range(KT // KG):
                if t == 0 and kg == 0:
                    xg = xg0
                else:
                    xg = xp.tile([128, KG, TCH], F32R, name="xg")
                    nc.sync.dma_start(
                        out=xg,
                        in_=x3[:, kg * KG:(kg + 1) * KG, t * TCH:(t + 1) * TCH])
                for ki in range(KG):
                    k = kg * KG + ki
                    for m in range(MT):
                        nc.tensor.matmul(
                            ps[m],
                            lhsT=w_sb[k][:, m * 128:(m + 1) * 128],
                            rhs=xg[:, ki, :],
                            start=(k == 0), stop=(k == KT - 1),
                        )
            cs = cos_sb[:, t * TCH:(t + 1) * TCH]
            sn = sin_sb[:, t * TCH:(t + 1) * TCH]
            for h in range(QH):
                _rope(nc, rp, ps[h], qT[:, h, t * TCH:(t + 1) * TCH], cs, sn)
            _rope(nc, rp, ps[QH], kT[:, t * TCH:(t + 1) * TCH], cs, sn)
            nc.scalar.copy(out=vT[:, t * TCH:(t + 1) * TCH], in_=ps[QH + 1])
            # V = vT.T for this chunk's kv tiles (PE transpose [d,tok]->[tok,d])
            for j in range(4 * t, 4 * t + 4):
                pv = pp1.tile([128, 128], F32, name="pvt", tag="pvt")
                nc.tensor.transpose(pv, vT[:, j * 128:(j + 1) * 128], ident_sb)
                nc.scalar.copy(out=V[:, j, :], in_=pv)

    # ---- phase 2: attention, per-head AllGather ------------------------
    # ---- phase 3: o_proj on gathered bf16 attention --------------------
    with (
        tc.tile_pool(name="pt", bufs=4) as ptp,
        tc.tile_pool(name="ao", bufs=2) as aop,
        tc.tile_pool(name="ow", bufs=16) as owp,
        tc.tile_pool(name="at", bufs=2) as atp,
        tc.tile_pool(name="acc", bufs=1) as accp,
        tc.tile_pool(name="p2sc", bufs=2, space="PSUM") as pp2,
        tc.tile_pool(name="p2acc", bufs=2, space="PSUM") as pa2,
    ):
        # o_proj output accumulator: acc[:, b, :] = out rows b*128:(b+1)*128
        acc = accp.tile([128, S // 128, TCH], F32)
        ow3 = ow.rearrange("(k p) d -> p k d", p=128)
        ag3 = [attn_gat[h].rearrange("(r p) s -> p r s", p=128)
               for h in range(QH)]
        out3 = out.rearrange("(b p) d -> p b d", p=128)

        def attention_chunk(h, c):
            """One (head, q-chunk), pipelined: QK(p+1) issues before PV(p)."""
            jmax = 4 * c + 3
            npair = (jmax + 1) // 2
            po = pa2.tile([128, TCH], F32, name="po", tag="po")
            psum_s = pa2.tile([128, TCH], F32, name="ps", tag="ps")
            qslice = qT[:, h, c * TCH:(c + 1) * TCH]

            def issue_qk(p):
                sc = pp2.tile([128, 2, TCH], F32, name="sc", tag="sc")
                for i in range(2):
                    j = 2 * p + i
                    nc.tensor.matmul(
                        sc[:, i, :], lhsT=kT[:, j * 128:(j + 1) * 128],
                        rhs=qslice, start=True, stop=True)
                pt = ptp.tile([128, 2, TCH], F32R, name="pt", tag="pt")
                nc.scalar.activation(pt, sc, EXP, scale=SCALE)
                for i in range(2):
                    j = 2 * p + i
                    rdiag = j - 4 * c
                    if rdiag >= 0:  # tile touches the causal diagonal
                        off = 384 - rdiag * 128
                        nc.gpsimd.tensor_mul(
                            pt[:, i, :], pt[:, i, :],
                            stair_sb[:, off:off + TCH])
                return pt

            def issue_pv(p, pt):
                for i in range(2):
                    j = 2 * p + i
                    nc.tensor.matmul(po, lhsT=V[:, j, :], rhs=pt[:, i, :],
                                     start=(j == 0), stop=(j == jmax))
                    nc.tensor.matmul(psum_s, lhsT=ones_sb, rhs=pt[:, i, :],
                                     start=(j == 0), stop=(j == jmax))

            prev = None
            for p in range(npair):
                pt = issue_qk(p)
                if prev is not None:
                    issue_pv(*prev)
                prev = (p, pt)
            issue_pv(*prev)
            rec = aop.tile([128, TCH], F32, name="rec")
            nc.vector.reciprocal(rec, psum_s)
            ao = aop.tile([128, TCH], BF16, name="ao")
            nc.vector.tensor_mul(ao, po, rec)
            # scalar-engine DMA queue: keeps the AllGather's input writes
            # off the congested sync queue so the gather triggers promptly
            nc.scalar.dma_start(
                out=attn_loc[h][:, c * TCH:(c + 1) * TCH], in_=ao)

        def allgather_head(h):
            nc.gpsimd.collective_compute(
                "AllGather",
                mybir.AluOpType.bypass,
                ins=[attn_loc[h][:, :]],
                outs=[attn_gat[h][:, :]],
                replica_groups=[list(range(NCORES))],
            )

        def oproj_load_weights(h):
            ows = []
            for r in range(NCORES):
                owk = owp.tile([128, DQ], BF16, name="owk", tag="owk")
                nc.sync.dma_start(out=owk, in_=ow3[:, r * QH + h, :])
                ows.append(owk)
            return ows

        def oproj_chunk(h, g, ows):
            """acc[:, 4g:4g+4, :] += sum_r at(r, h) @ ow(r, h) for 512 toks."""
            at = atp.tile([128, NCORES, TCH], BF16, name="at", tag="at")
            nc.sync.dma_start(
                out=at, in_=ag3[h][:, :, g * TCH:(g + 1) * TCH])
            for mp in range(2):
                pcs = [pp2.tile([128, TCH], F32, name=f"pc{i}", tag="sc")
                       for i in range(2)]
                for r in range(NCORES):
                    for i, mi in enumerate((2 * mp, 2 * mp + 1)):
                        nc.tensor.matmul(
                            pcs[i],
                            lhsT=at[:, r, mi * 128:(mi + 1) * 128],
                            rhs=ows[r],
                            start=(r == 0), stop=(r == NCORES - 1),
                        )
                for i, mi in enumerate((2 * mp, 2 * mp + 1)):
                    b = g * 4 + mi
                    if h == 0:
                        nc.scalar.copy(out=acc[:, b, :], in_=pcs[i])
                    else:
                        nc.vector.tensor_add(acc[:, b, :], acc[:, b, :],
                                             pcs[i])
            if h == QH - 1:
                nc.sync.dma_start(out=out3[:, 4 * g:4 * g + 4, :],
                                  in_=acc[:, 4 * g:4 * g + 4, :])

        for h in range(QH):
            for c in range(NTCH):
                attention_chunk(h, c)
            allgather_head(h)
        for h in range(QH):
            ows = oproj_load_weights(h)
            for g in range(NTCH):
                oproj_chunk(h, g, ows)


_NC_CACHE = None


def build_program():
    global _NC_CACHE
    if _NC_CACHE is not None:
        return _NC_CACHE
    nc = bacc.Bacc("TRN2", target_bir_lowering=False, debug=False,
                   num_devices=NCORES)
    ins = {
        "xT": nc.dram_tensor("xT", [HIDDEN, S], F32R, kind="ExternalInput").ap(),
        "wqkv": nc.dram_tensor("wqkv", [HIDDEN, DOUT], F32R,
                               kind="ExternalInput").ap(),
        "ow": nc.dram_tensor("ow", [HIDDEN, DQ], BF16, kind="ExternalInput").ap(),
        "cos_t": nc.dram_tensor("cos_t", [64, S], F32, kind="ExternalInput").ap(),
        "sin_t": nc.dram_tensor("sin_t", [64, S], F32, kind="ExternalInput").ap(),
        "stair": nc.dram_tensor("stair", [128, 896], F32,
                                kind="ExternalInput").ap(),
    }
    outs = {"out": nc.dram_tensor("out", [S, DQ], F32, kind="ExternalOutput").ap()}
    with tile.TileContext(nc) as tc:
        with ExitStack() as ctx:
            build_kernel_body(ctx, tc, outs, ins)
    nc.compile()
    _NC_CACHE = nc
    return nc


def make_in_maps(hidden_states, position_ids, q_w, k_w, v_w, o_w):
    import ml_dtypes

    x = np.asarray(hidden_states, dtype=np.float32).reshape(S, HIDDEN)
    xT = np.ascontiguousarray(x.T)
    pos = np.asarray(position_ids).reshape(S).astype(np.float64)
    inv = 1.0 / (THETA ** (np.arange(0, HD, 2, dtype=np.float64) / HD))
    fr = inv[:, None] * pos[None, :]                       # [64, S]
    cos_t = np.cos(fr).astype(np.float32)
    sin_t = np.sin(fr).astype(np.float32)
    u = np.arange(896, dtype=np.int64)[None, :]
    kvi = np.arange(128, dtype=np.int64)[:, None]
    stair = ((u - kvi) >= 384).astype(np.float32)          # [128, 896]

    q_w = np.asarray(q_w, dtype=np.float32)
    k_w = np.asarray(k_w, dtype=np.float32)
    v_w = np.asarray(v_w, dtype=np.float32)
    o_w = np.asarray(o_w, dtype=np.float32)

    in_maps = []
    for c in range(NCORES):
        wqkv = np.ascontiguousarray(np.concatenate(
            [q_w[:, c * DQ:(c + 1) * DQ],
             k_w[:, c * HD:(c + 1) * HD],
             v_w[:, c * HD:(c + 1) * HD]], axis=1))
        owc = np.ascontiguousarray(
            o_w[:, c * DQ:(c + 1) * DQ].astype(ml_dtypes.bfloat16))
        in_maps.append({"xT": xT, "wqkv": wqkv, "ow": owc,
                        "cos_t": cos_t, "sin_t": sin_t, "stair": stair})
    return in_maps


def run(inputs: dict, trace: bool = False):
    """Run on the 8 NeuronCores; returns (full_output, BassKernelResults)."""
    nc = build_program()
    in_maps = make_in_maps(**inputs)
    res = run_bass_kernel_spmd(nc, in_maps, core_ids=list(range(NCORES)),
                               trace=trace)
    full = np.concatenate([res.results[c]["out"] for c in range(NCORES)], axis=1)
    return full.reshape(1, S, HIDDEN), res


def kernel(**inputs) -> np.ndarray:
    out, _ = run(inputs)
    return out


# revision 13
# speedup vs baseline: 1.1701x; 1.0728x over previous
# Mistral sliding-window attention (B=1, S=2048, H=4096, 32 q heads / 8 kv
# heads, window 4096 -> plain causal at this S) on 8 Trainium2 NeuronCores.
#
# Sharding: tensor-parallel over heads. Core c owns q heads 4c..4c+3 and kv
# head c. hidden_states is replicated (transposed on host to [H, S] so the
# contraction dim is the partition dim). Each core computes its attention
# output slice, per-head AllGathers (bf16) assemble the full attention while
# later heads still compute, and each core computes a 512-column slice of
# o_proj; the host concatenates the 8 column slices.
#
# v2 notes:
# - QKV path stays fp32 (float32r matmuls run full-rate at moving dim 512);
#   the o_proj path (attention outs -> AllGather -> gather reads -> o_w) is
#   bf16, halving collective+DMA bytes there at ~0.4% quantization error.
# - Causal mask is a 0/1 staircase multiplied into the probabilities
#   post-exp on the otherwise-idle gpsimd engine (psum bias pre-fill would
#   avoid the dependency but matmul start=False onto engine-written psum is
#   rejected by the accumulation-group model).
# - Attention is software-pipelined with one j-pair of lookahead: the PE
#   issues QK(p+1) before PV(p), hiding the ACT-engine exp latency.
# - attn_loc writes go out on the scalar engine's DMA queue; the bulky
#   loads (weights, x, gathered attention) stay on the sync queue. The
#   per-head AllGather trigger waits only on the scalar queue's few ao
#   writes, so it fires as soon as that head is done (the v1 kernel's
#   gathers all serialized after the whole attention phase because the
#   gather-input writes sat behind stalled gather-output reads in one
#   in-order queue).

from contextlib import ExitStack

import numpy as np

import concourse.bacc as bacc
import concourse.bass as bass
import concourse.mybir as mybir
import concourse.tile as tile
from concourse.bass_utils import run_bass_kernel_spmd
from concourse.masks import make_identity

HIDDEN = 4096
NH = 32
NKV = 8
HD = 128
THETA = 10000.0
S = 2048
NCORES = 8

QH = NH // NCORES          # 4 q heads per core
DQ = QH * HD               # 512 (per-core q/attn width)
DOUT = DQ + 2 * HD         # 768 = q heads + k + v projection width
MT = DOUT // 128           # 6 projection m-tiles (0..3 q, 4 k, 5 v)
KT = HIDDEN // 128         # 32 contraction tiles
KG = 4                     # x-load group: k-tiles per DMA
TCH = 512                  # token chunk (matmul moving dim)
NTCH = S // TCH            # 4
KVT = S // 128             # 16 kv tiles
SCALE = 1.0 / float(np.sqrt(HD))


F32 = mybir.dt.float32
F32R = mybir.dt.float32r
BF16 = mybir.dt.bfloat16
EXP = mybir.ActivationFunctionType.Exp


def _rope(nc, pool, src, dst, cs, sn):
    """dst = src*cos + rotate_half(src)*sin, in [d, tok] layout.

    src/dst are [128, n]; cs/sn are [64, n] (the two 64-row halves share
    frequencies). rotate_half: rows 0:64 get -src[64:128], rows 64:128 get
    src[0:64].
    """
    top, bot = src[0:64, :], src[64:128, :]
    ta = pool.tile([64, TCH], F32, name="rope_a")
    tb = pool.tile([64, TCH], F32, name="rope_b")
    nc.vector.tensor_mul(ta, top, cs)
    nc.vector.tensor_mul(tb, bot, sn)
    nc.vector.tensor_sub(dst[0:64, :], ta, tb)
    nc.vector.tensor_mul(ta, bot, cs)
    nc.vector.tensor_mul(tb, top, sn)
    nc.vector.tensor_add(dst[64:128, :], ta, tb)


def build_kernel_body(ctx: ExitStack, tc: tile.TileContext, outs, ins):
    nc = tc.nc
    xT, wqkv, ow, cos_t, sin_t, stair = (
        ins["xT"], ins["wqkv"], ins["ow"], ins["cos_t"], ins["sin_t"],
        ins["stair"],
    )
    out = outs["out"]

    # per-head bounce + gather buffers so each head's AllGather can fire as
    # soon as that head's attention is done (overlaps comm with compute)
    attn_loc = [nc.dram_tensor(f"attn_loc{h}", [HD, S], BF16).ap()
                for h in range(QH)]
    attn_gat = [nc.dram_tensor(f"attn_gat{h}", [NCORES * HD, S], BF16,
                               addr_space="Shared").ap()
                for h in range(QH)]

    singles = ctx.enter_context(tc.tile_pool(name="singles", bufs=1))
    stair_sb = singles.tile([128, 896], F32)
    nc.sync.dma_start(out=stair_sb, in_=stair)
    ones_sb = singles.tile([128, 128], F32R)

    # persistent projection outputs, [d, tok] layout
    qT = singles.tile([128, QH, S], F32R)    # q head h -> qT[:, h, :]
    kT = singles.tile([128, S], F32R)
    V = singles.tile([128, KVT, HD], F32R)   # V[:, j, :] = [tok 128, d 128]

    # ---- phase 1: QKV projection + RoPE --------------------------------
    with (
        tc.tile_pool(name="wq", bufs=1) as wp,
        tc.tile_pool(name="xt", bufs=3) as xp,
        tc.tile_pool(name="rope", bufs=2) as rp,
        tc.tile_pool(name="p1ps", bufs=1, space="PSUM") as pp1,
    ):
        cos_sb = wp.tile([64, S], F32)
        sin_sb = wp.tile([64, S], F32)
        vT = wp.tile([128, S], F32)
        ident_sb = wp.tile([128, 128], F32)
        ones_f = wp.tile([128, 128], F32)
        nc.vector.memset(ones_f, 1.0)
        nc.vector.tensor_copy(ones_sb, ones_f)
        make_identity(nc, ident_sb)

        wq3 = wqkv.rearrange("(k p) d -> p k d", p=128)
        x3 = xT.rearrange("(k p) s -> p k s", p=128)
        # x chunk (t=0, kg=0) first so the first matmul starts almost
        # immediately; weight k-tiles follow in per-tile DMAs
        w_sb = [wp.tile([128, DOUT], F32R, name=f"w{k}", tag=f"w{k}")
                for k in range(KT)]
        xg0 = xp.tile([128, KG, TCH], F32R, name="xg")
        nc.sync.dma_start(out=xg0, in_=x3[:, 0:KG, 0:TCH])
        # weights stream on the scalar engine's DMA queue, x chunks on the
        # sync queue: chunk 0 needs ~20MB in its ~45us of matmuls, more
        # than one queue's bandwidth
        for k in range(KT):
            nc.scalar.dma_start(out=w_sb[k], in_=wq3[:, k, :])
        nc.sync.dma_start(out=cos_sb, in_=cos_t)
        nc.sync.dma_start(out=sin_sb, in_=sin_t)
        for t in range(NTCH):
            ps = [pp1.tile([128, TCH], F32, name=f"p1_{m}", tag=f"p1_{m}")
                  for m in range(MT)]
            for kg in range(KT // KG):
                if t == 0 and kg == 0:
                    xg = xg0
                else:
                    xg = xp.tile([128, KG, TCH], F32R, name="xg")
                    nc.sync.dma_start(
                        out=xg,
                        in_=x3[:, kg * KG:(kg + 1) * KG, t * TCH:(t + 1) * TCH])
                for ki in range(KG):
                    k = kg * KG + ki
                    for m in range(MT):
                        nc.tensor.matmul(
                            ps[m],
                            lhsT=w_sb[k][:, m * 128:(m + 1) * 128],
                            rhs=xg[:, ki, :],
                            start=(k == 0), stop=(k == KT - 1),
                        )
            cs = cos_sb[:, t * TCH:(t + 1) * TCH]
            sn = sin_sb[:, t * TCH:(t + 1) * TCH]
            for h in range(QH):
                _rope(nc, rp, ps[h], qT[:, h, t * TCH:(t + 1) * TCH], cs, sn)
            _rope(nc, rp, ps[QH], kT[:, t * TCH:(t + 1) * TCH], cs, sn)
            nc.scalar.copy(out=vT[:, t * TCH:(t + 1) * TCH], in_=ps[QH + 1])
            # V = vT.T for this chunk's kv tiles (PE transpose [d,tok]->[tok,d])
            for j in range(4 * t, 4 * t + 4):
                pv = pp1.tile([128, 128], F32, name="pvt", tag="pvt")
                nc.tensor.transpose(pv, vT[:, j * 128:(j + 1) * 128], ident_sb)
                nc.scalar.copy(out=V[:, j, :], in_=pv)

    # ---- phase 2: attention, per-head AllGather ------------------------
    # ---- phase 3: o_proj on gathered bf16 attention --------------------
    with (
        tc.tile_pool(name="pt", bufs=4) as ptp,
        tc.tile_pool(name="ao", bufs=2) as aop,
        tc.tile_pool(name="ow", bufs=16) as owp,
        tc.tile_pool(name="at", bufs=2) as atp,
        tc.tile_pool(name="acc", bufs=1) as accp,
        tc.tile_pool(name="p2sc", bufs=2, space="PSUM") as pp2,
        tc.tile_pool(name="p2acc", bufs=2, space="PSUM") as pa2,
    ):
        # o_proj output accumulator: acc[:, b, :] = out rows b*128:(b+1)*128
        acc = accp.tile([128, S // 128, TCH], F32)
        ow3 = ow.rearrange("(k p) d -> p k d", p=128)
        ag3 = [attn_gat[h].rearrange("(r p) s -> p r s", p=128)
               for h in range(QH)]
        out3 = out.rearrange("(b p) d -> p b d", p=128)

        def attention_chunk(h, c):
            """One (head, q-chunk), pipelined: QK(p+1) issues before PV(p)."""
            jmax = 4 * c + 3
            npair = (jmax + 1) // 2
            po = pa2.tile([128, TCH], F32, name="po", tag="po")
            psum_s = pa2.tile([128, TCH], F32, name="ps", tag="ps")
            qslice = qT[:, h, c * TCH:(c + 1) * TCH]

            def issue_qk(p):
                sc = pp2.tile([128, 2, TCH], F32, name="sc", tag="sc")
                for i in range(2):
                    j = 2 * p + i
                    nc.tensor.matmul(
                        sc[:, i, :], lhsT=kT[:, j * 128:(j + 1) * 128],
                        rhs=qslice, start=True, stop=True)
                pt = ptp.tile([128, 2, TCH], F32R, name="pt", tag="pt")
                nc.scalar.activation(pt, sc, EXP, scale=SCALE)
                for i in range(2):
                    j = 2 * p + i
                    rdiag = j - 4 * c
                    if rdiag >= 0:  # tile touches the causal diagonal
                        off = 384 - rdiag * 128
                        nc.vector.tensor_mul(
                            pt[:, i, :], pt[:, i, :],
                            stair_sb[:, off:off + TCH])
                return pt

            def issue_pv(p, pt):
                for i in range(2):
                    j = 2 * p + i
                    nc.tensor.matmul(po, lhsT=V[:, j, :], rhs=pt[:, i, :],
                                     start=(j == 0), stop=(j == jmax))
                    nc.tensor.matmul(psum_s, lhsT=ones_sb, rhs=pt[:, i, :],
                                     start=(j == 0), stop=(j == jmax))

            prev = None
            for p in range(npair):
                pt = issue_qk(p)
                if prev is not None:
                    issue_pv(*prev)
                prev = (p, pt)
            issue_pv(*prev)
            rec = aop.tile([128, TCH], F32, name="rec")
            nc.vector.reciprocal(rec, psum_s)
            ao = aop.tile([128, TCH], BF16, name="ao")
            nc.vector.tensor_mul(ao, po, rec)
            # scalar-engine DMA queue: keeps the AllGather's input writes
            # off the congested sync queue so the gather triggers promptly
            nc.scalar.dma_start(
                out=attn_loc[h][:, c * TCH:(c + 1) * TCH], in_=ao)

        def allgather_head(h):
            nc.gpsimd.collective_compute(
                "AllGather",
                mybir.AluOpType.bypass,
                ins=[attn_loc[h][:, :]],
                outs=[attn_gat[h][:, :]],
                replica_groups=[list(range(NCORES))],
            )

        def oproj_load_weights(h):
            ows = []
            for r in range(NCORES):
                owk = owp.tile([128, DQ], BF16, name="owk", tag="owk")
                nc.sync.dma_start(out=owk, in_=ow3[:, r * QH + h, :])
                ows.append(owk)
            return ows

        def oproj_chunk(h, g, ows):
            """acc[:, 4g:4g+4, :] += sum_r at(r, h) @ ow(r, h) for 512 toks."""
            at = atp.tile([128, NCORES, TCH], BF16, name="at", tag="at")
            nc.sync.dma_start(
                out=at, in_=ag3[h][:, :, g * TCH:(g + 1) * TCH])
            for mp in range(2):
                pcs = [pp2.tile([128, TCH], F32, name=f"pc{i}", tag="sc")
                       for i in range(2)]
                for r in range(NCORES):
                    for i, mi in enumerate((2 * mp, 2 * mp + 1)):
                        nc.tensor.matmul(
                            pcs[i],
                            lhsT=at[:, r, mi * 128:(mi + 1) * 128],
                            rhs=ows[r],
                            start=(r == 0), stop=(r == NCORES - 1),
                        )
                for i, mi in enumerate((2 * mp, 2 * mp + 1)):
                    b = g * 4 + mi
                    if h == 0:
                        nc.scalar.copy(out=acc[:, b, :], in_=pcs[i])
                    else:
                        nc.vector.tensor_add(acc[:, b, :], acc[:, b, :],
                                             pcs[i])
            if h == QH - 1:
                nc.sync.dma_start(out=out3[:, 4 * g:4 * g + 4, :],
                                  in_=acc[:, 4 * g:4 * g + 4, :])

        for h in range(QH):
            for c in range(NTCH):
                attention_chunk(h, c)
            allgather_head(h)
        for h in range(QH):
            ows = oproj_load_weights(h)
            for g in range(NTCH):
                oproj_chunk(h, g, ows)


_NC_CACHE = None


def build_program():
    global _NC_CACHE
    if _NC_CACHE is not None:
        return _NC_CACHE
    nc = bacc.Bacc("TRN2", target_bir_lowering=False, debug=False,
                   num_devices=NCORES)
    ins = {
        "xT": nc.dram_tensor("xT", [HIDDEN, S], F32R, kind="ExternalInput").ap(),
        "wqkv": nc.dram_tensor("wqkv", [HIDDEN, DOUT], F32R,
                               kind="ExternalInput").ap(),
        "ow": nc.dram_tensor("ow", [HIDDEN, DQ], BF16, kind="ExternalInput").ap(),
        "cos_t": nc.dram_tensor("cos_t", [64, S], F32, kind="ExternalInput").ap(),
        "sin_t": nc.dram_tensor("sin_t", [64, S], F32, kind="ExternalInput").ap(),
        "stair": nc.dram_tensor("stair", [128, 896], F32,
                                kind="ExternalInput").ap(),
    }
    outs = {"out": nc.dram_tensor("out", [S, DQ], F32, kind="ExternalOutput").ap()}
    with tile.TileContext(nc) as tc:
        with ExitStack() as ctx:
            build_kernel_body(ctx, tc, outs, ins)
    nc.compile()
    _NC_CACHE = nc
    return nc


def make_in_maps(hidden_states, position_ids, q_w, k_w, v_w, o_w):
    import ml_dtypes

    x = np.asarray(hidden_states, dtype=np.float32).reshape(S, HIDDEN)
    xT = np.ascontiguousarray(x.T)
    pos = np.asarray(position_ids).reshape(S).astype(np.float64)
    inv = 1.0 / (THETA ** (np.arange(0, HD, 2, dtype=np.float64) / HD))
    fr = inv[:, None] * pos[None, :]                       # [64, S]
    cos_t = np.cos(fr).astype(np.float32)
    sin_t = np.sin(fr).astype(np.float32)
    u = np.arange(896, dtype=np.int64)[None, :]
    kvi = np.arange(128, dtype=np.int64)[:, None]
    stair = ((u - kvi) >= 384).astype(np.float32)          # [128, 896]

    q_w = np.asarray(q_w, dtype=np.float32)
    k_w = np.asarray(k_w, dtype=np.float32)
    v_w = np.asarray(v_w, dtype=np.float32)
    o_w = np.asarray(o_w, dtype=np.float32)

    in_maps = []
    for c in range(NCORES):
        wqkv = np.ascontiguousarray(np.concatenate(
            [q_w[:, c * DQ:(c + 1) * DQ],
             k_w[:, c * HD:(c + 1) * HD],
             v_w[:, c * HD:(c + 1) * HD]], axis=1))
        owc = np.ascontiguousarray(
            o_w[:, c * DQ:(c + 1) * DQ].astype(ml_dtypes.bfloat16))
        in_maps.append({"xT": xT, "wqkv": wqkv, "ow": owc,
                        "cos_t": cos_t, "sin_t": sin_t, "stair": stair})
    return in_maps


def run(inputs: dict, trace: bool = False):
    """Run on the 8 NeuronCores; returns (full_output, BassKernelResults)."""
    nc = build_program()
    in_maps = make_in_maps(**inputs)
    res = run_bass_kernel_spmd(nc, in_maps, core_ids=list(range(NCORES)),
                               trace=trace)
    full = np.concatenate([res.results[c]["out"] for c in range(NCORES)], axis=1)
    return full.reshape(1, S, HIDDEN), res


def kernel(**inputs) -> np.ndarray:
    out, _ = run(inputs)
    return out


# revision 19
# speedup vs baseline: 1.2044x; 1.0294x over previous
# Mistral sliding-window attention (B=1, S=2048, H=4096, 32 q heads / 8 kv
# heads, window 4096 -> plain causal at this S) on 8 Trainium2 NeuronCores.
#
# Sharding: tensor-parallel over heads. Core c owns q heads 4c..4c+3 and kv
# head c. hidden_states is replicated (transposed on host to [H, S] so the
# contraction dim is the partition dim). Each core computes its attention
# output slice, per-head AllGathers (bf16) assemble the full attention while
# later heads still compute, and each core computes a 512-column slice of
# o_proj; the host concatenates the 8 column slices.
#
# v2 notes:
# - QKV path stays fp32 (float32r matmuls run full-rate at moving dim 512);
#   the o_proj path (attention outs -> AllGather -> gather reads -> o_w) is
#   bf16, halving collective+DMA bytes there at ~0.4% quantization error.
# - Weight tiles 4..31 stream on the scalar engine's DMA queue, everything
#   else on the sync queue: chunk 0 needs ~20MB inside its ~45us of
#   matmuls, more than one queue's bandwidth.
# - qT/kT/V live in per-chunk tiles: tile-granularity dependency tracking
#   would otherwise make the first attention matmul wait for the LAST
#   chunk's rope writes.
# - Attention is one flat software-pipelined stream over (head, chunk,
#   j-pair) with 2 pairs of lookahead, so the PE crosses chunk/head
#   boundaries without waiting for the exp -> mask chain of the last pair.
# - attn_loc writes go out on the scalar engine's DMA queue so the
#   per-head AllGather trigger fires promptly instead of behind the
#   sync queue's bulk traffic.

from contextlib import ExitStack

import numpy as np

import concourse.bacc as bacc
import concourse.bass as bass
import concourse.mybir as mybir
import concourse.tile as tile
from concourse.bass_utils import run_bass_kernel_spmd
from concourse.masks import make_identity

HIDDEN = 4096
NH = 32
NKV = 8
HD = 128
THETA = 10000.0
S = 2048
NCORES = 8

QH = NH // NCORES          # 4 q heads per core
DQ = QH * HD               # 512 (per-core q/attn width)
DOUT = DQ + 2 * HD         # 768 = q heads + k + v projection width
MT = DOUT // 128           # 6 projection m-tiles (0..3 q, 4 k, 5 v)
KT = HIDDEN // 128         # 32 contraction tiles
KG = 4                     # x-load group: k-tiles per DMA
TCH = 512                  # token chunk (matmul moving dim)
NTCH = S // TCH            # 4
KVT = S // 128             # 16 kv tiles
SCALE = 1.0 / float(np.sqrt(HD))

F32 = mybir.dt.float32
F32R = mybir.dt.float32r
BF16 = mybir.dt.bfloat16
EXP = mybir.ActivationFunctionType.Exp


def _rope(nc, pool, src, dst, cs, sn):
    """dst = src*cos + rotate_half(src)*sin, in [d, tok] layout.

    src/dst are [128, n]; cs/sn are [64, n] (the two 64-row halves share
    frequencies). rotate_half: rows 0:64 get -src[64:128], rows 64:128 get
    src[0:64].
    """
    top, bot = src[0:64, :], src[64:128, :]
    ta = pool.tile([64, TCH], F32, name="rope_a")
    tb = pool.tile([64, TCH], F32, name="rope_b")
    nc.vector.tensor_mul(ta, top, cs)
    nc.vector.tensor_mul(tb, bot, sn)
    nc.vector.tensor_sub(dst[0:64, :], ta, tb)
    nc.vector.tensor_mul(ta, bot, cs)
    nc.vector.tensor_mul(tb, top, sn)
    nc.vector.tensor_add(dst[64:128, :], ta, tb)


def build_kernel_body(ctx: ExitStack, tc: tile.TileContext, outs, ins):
    nc = tc.nc
    xT, wqkv, ow, cos_t, sin_t, stair = (
        ins["xT"], ins["wqkv"], ins["ow"], ins["cos_t"], ins["sin_t"],
        ins["stair"],
    )
    out = outs["out"]

    # per-head bounce + gather buffers so each head's AllGather can fire as
    # soon as that head's attention is done (overlaps comm with compute)
    attn_loc = [nc.dram_tensor(f"attn_loc{h}", [HD, S], BF16).ap()
                for h in range(QH)]
    attn_gat = [nc.dram_tensor(f"attn_gat{h}", [NCORES * HD, S], BF16,
                               addr_space="Shared").ap()
                for h in range(QH)]

    singles = ctx.enter_context(tc.tile_pool(name="singles", bufs=1))
    stair_sb = singles.tile([128, 896], F32)
    ones_sb = singles.tile([128, 128], F32R)

    # per-chunk projection outputs, [d, tok] layout (separate tiles per
    # chunk so attention's dependencies stay chunk-granular)
    qTt = [singles.tile([128, QH, TCH], F32R, name=f"qT{t}")
           for t in range(NTCH)]
    kTt = [singles.tile([128, TCH], F32R, name=f"kT{t}")
           for t in range(NTCH)]
    Vt = [singles.tile([128, 4, HD], F32R, name=f"V{t}")
          for t in range(NTCH)]

    def kslice(j):  # kv tile j of roped K, [128 d, 128 tok]
        return kTt[j // 4][:, (j % 4) * 128:(j % 4 + 1) * 128]

    def vtile(j):   # kv tile j of V, [128 tok, 128 d]
        return Vt[j // 4][:, j % 4, :]

    # ---- phase 1: QKV projection + RoPE --------------------------------
    with (
        tc.tile_pool(name="wq", bufs=1) as wp,
        tc.tile_pool(name="xt", bufs=3) as xp,
        tc.tile_pool(name="qk", bufs=2) as qkp,
        tc.tile_pool(name="rope", bufs=2) as rp,
        tc.tile_pool(name="p1ps", bufs=1, space="PSUM") as pp1,
    ):
        cos_sb = wp.tile([64, S], F32)
        sin_sb = wp.tile([64, S], F32)
        ident_sb = wp.tile([128, 128], F32)
        ones_f = wp.tile([128, 128], F32)

        wq3 = wqkv.rearrange("(k p) d -> p k d", p=128)
        x3 = xT.rearrange("(k p) s -> p k s", p=128)
        # first x chunk + first weight tiles on the sync queue so the PE
        # starts immediately; the remaining weight tiles stream on the
        # scalar engine's DMA queue in parallel
        w_sb = [wp.tile([128, DOUT], F32R, name=f"w{k}", tag=f"w{k}")
                for k in range(KT)]
        xg0 = xp.tile([128, KG, TCH], F32R, name="xg")
        nc.sync.dma_start(out=xg0, in_=x3[:, 0:KG, 0:TCH])
        for k in range(KT):
            eng = nc.sync if k % 2 == 0 else nc.scalar
            eng.dma_start(out=w_sb[k], in_=wq3[:, k, :])
        nc.sync.dma_start(out=cos_sb, in_=cos_t)
        nc.sync.dma_start(out=sin_sb, in_=sin_t)
        nc.sync.dma_start(out=stair_sb, in_=stair)
        nc.vector.memset(ones_f, 1.0)
        nc.vector.tensor_copy(ones_sb, ones_f)
        make_identity(nc, ident_sb)

        for t in range(NTCH):
            ps = [pp1.tile([128, TCH], F32, name=f"p1_{m}", tag=f"p1_{m}")
                  for m in range(MT)]
            for kg in range(KT // KG):
                if t == 0 and kg == 0:
                    xg = xg0
                else:
                    xg = xp.tile([128, KG, TCH], F32R, name="xg")
                    nc.sync.dma_start(
                        out=xg,
                        in_=x3[:, kg * KG:(kg + 1) * KG, t * TCH:(t + 1) * TCH])
                for ki in range(KG):
                    k = kg * KG + ki
                    for m in range(MT):
                        nc.tensor.matmul(
                            ps[m],
                            lhsT=w_sb[k][:, m * 128:(m + 1) * 128],
                            rhs=xg[:, ki, :],
                            start=(k == 0), stop=(k == KT - 1),
                        )
            # ACT copies the V psum -> SBUF; PE transposes it to [tok, d]
            vT = qkp.tile([128, TCH], F32, name="vT")
            nc.scalar.copy(out=vT, in_=ps[QH + 1])
            for jj in range(4):
                pv = pp1.tile([128, 128], F32, name="pvt", tag="pvt")
                nc.tensor.transpose(pv, vT[:, jj * 128:(jj + 1) * 128],
                                    ident_sb)
                nc.scalar.copy(out=Vt[t][:, jj, :], in_=pv)
            cs = cos_sb[:, t * TCH:(t + 1) * TCH]
            sn = sin_sb[:, t * TCH:(t + 1) * TCH]
            for h in range(QH):
                _rope(nc, rp, ps[h], qTt[t][:, h, :], cs, sn)
            _rope(nc, rp, ps[QH], kTt[t], cs, sn)

    # ---- phase 2: attention, per-head AllGather ------------------------
    # ---- phase 3: o_proj on gathered bf16 attention --------------------
    with (
        tc.tile_pool(name="pt", bufs=4) as ptp,
        tc.tile_pool(name="ao", bufs=2) as aop,
        tc.tile_pool(name="ow", bufs=16) as owp,
        tc.tile_pool(name="at", bufs=2) as atp,
        tc.tile_pool(name="acc", bufs=1) as accp,
        tc.tile_pool(name="p2sc", bufs=2, space="PSUM") as pp2,
        tc.tile_pool(name="p2po", bufs=2, space="PSUM") as pop,
        tc.tile_pool(name="p2ps", bufs=2, space="PSUM") as psp,
    ):
        # o_proj output accumulator: acc[:, b, :] = out rows b*128:(b+1)*128
        acc = accp.tile([128, S // 128, TCH], F32)
        ow3 = ow.rearrange("(k p) d -> p k d", p=128)
        ag3 = [attn_gat[h].rearrange("(r p) s -> p r s", p=128)
               for h in range(QH)]
        out3 = out.rearrange("(b p) d -> p b d", p=128)

        def issue_qk(h, c, p):
            sc = pp2.tile([128, 2, TCH], F32, name="sc", tag="sc")
            qslice = qTt[c][:, h, :]
            for i in range(2):
                j = 2 * p + i
                nc.tensor.matmul(sc[:, i, :], lhsT=kslice(j), rhs=qslice,
                                 start=True, stop=True)
            pt = ptp.tile([128, 2, TCH], F32R, name="pt", tag="pt")
            nc.scalar.activation(pt, sc, EXP, scale=SCALE)
            for i in range(2):
                j = 2 * p + i
                rdiag = j - 4 * c
                if rdiag >= 0:  # tile touches the causal diagonal
                    off = 384 - rdiag * 128
                    nc.vector.tensor_mul(pt[:, i, :], pt[:, i, :],
                                         stair_sb[:, off:off + TCH])
            return pt

        def issue_pv(st, p, pt):
            jmax = 4 * st["c"] + 3
            for i in range(2):
                j = 2 * p + i
                nc.tensor.matmul(st["po"], lhsT=vtile(j), rhs=pt[:, i, :],
                                 start=(j == 0), stop=(j == jmax))
                nc.tensor.matmul(st["ps"], lhsT=ones_sb, rhs=pt[:, i, :],
                                 start=(j == 0), stop=(j == jmax))

        def finalize_chunk(st):
            h, c = st["h"], st["c"]
            rec = aop.tile([128, TCH], F32, name="rec")
            nc.vector.reciprocal(rec, st["ps"])
            ao = aop.tile([128, TCH], BF16, name="ao")
            nc.vector.tensor_mul(ao, st["po"], rec)
            # scalar-engine DMA queue: keeps the AllGather's input writes
            # off the congested sync queue so the gather triggers promptly
            nc.scalar.dma_start(
                out=attn_loc[h][:, c * TCH:(c + 1) * TCH], in_=ao)

        def allgather_head(h):
            nc.gpsimd.collective_compute(
                "AllGather",
                mybir.AluOpType.bypass,
                ins=[attn_loc[h][:, :]],
                outs=[attn_gat[h][:, :]],
                replica_groups=[list(range(NCORES))],
            )

        # one flat software-pipelined stream over (head, chunk, j-pair):
        # PV lags QK by LOOK pairs so the PE never waits on exp/mask
        LOOK = 2
        seq = []
        for h in range(QH):
            for c in range(NTCH):
                st = {"h": h, "c": c, "po": None, "ps": None}
                for p in range(2 * (c + 1)):
                    seq.append((st, p))
        pending = []

        def pump(lim):
            while len(pending) > lim:
                st, p, pt = pending.pop(0)
                issue_pv(st, p, pt)
                if 2 * p + 1 == 4 * st["c"] + 3:  # last pair of the chunk
                    finalize_chunk(st)
                    if st["c"] == NTCH - 1:
                        allgather_head(st["h"])

        for st, p in seq:
            if st["po"] is None:
                st["po"] = pop.tile([128, TCH], F32, name="po", tag="po")
                st["ps"] = psp.tile([128, TCH], F32, name="ps", tag="ps")
            pt = issue_qk(st["h"], st["c"], p)
            pending.append((st, p, pt))
            pump(LOOK)
        pump(0)

        def oproj_load_weights(h):
            ows = []
            for r in range(NCORES):
                owk = owp.tile([128, DQ], BF16, name="owk", tag="owk")
                nc.sync.dma_start(out=owk, in_=ow3[:, r * QH + h, :])
                ows.append(owk)
            return ows

        def oproj_chunk(h, g, ows):
            """acc[:, 4g:4g+4, :] += sum_r at(r, h) @ ow(r, h) for 512 toks."""
            at = atp.tile([128, NCORES, TCH], BF16, name="at", tag="at")
            nc.sync.dma_start(
                out=at, in_=ag3[h][:, :, g * TCH:(g + 1) * TCH])
            for mp in range(2):
                pcs = pp2.tile([128, 2, TCH], F32, name="pc", tag="sc")
                for r in range(NCORES):
                    for i, mi in enumerate((2 * mp, 2 * mp + 1)):
                        nc.tensor.matmul(
                            pcs[:, i, :],
                            lhsT=at[:, r, mi * 128:(mi + 1) * 128],
                            rhs=ows[r],
                            start=(r == 0), stop=(r == NCORES - 1),
                        )
                for i, mi in enumerate((2 * mp, 2 * mp + 1)):
                    b = g * 4 + mi
                    if h == 0:
                        nc.scalar.copy(out=acc[:, b, :], in_=pcs[:, i, :])
                    else:
                        nc.vector.tensor_add(acc[:, b, :], acc[:, b, :],
                                             pcs[:, i, :])
            if h == QH - 1:
                nc.sync.dma_start(out=out3[:, 4 * g:4 * g + 4, :],
                                  in_=acc[:, 4 * g:4 * g + 4, :])

        for h in range(QH):
            ows = oproj_load_weights(h)
            for g in range(NTCH):
                oproj_chunk(h, g, ows)


_NC_CACHE = None


def build_program():
    global _NC_CACHE
    if _NC_CACHE is not None:
        return _NC_CACHE
    nc = bacc.Bacc("TRN2", target_bir_lowering=False, debug=False,
                   num_devices=NCORES)
    ins = {
        "xT": nc.dram_tensor("xT", [HIDDEN, S], F32R, kind="ExternalInput").ap(),
        "wqkv": nc.dram_tensor("wqkv", [HIDDEN, DOUT], F32R,
                               kind="ExternalInput").ap(),
        "ow": nc.dram_tensor("ow", [HIDDEN, DQ], BF16, kind="ExternalInput").ap(),
        "cos_t": nc.dram_tensor("cos_t", [64, S], F32, kind="ExternalInput").ap(),
        "sin_t": nc.dram_tensor("sin_t", [64, S], F32, kind="ExternalInput").ap(),
        "stair": nc.dram_tensor("stair", [128, 896], F32,
                                kind="ExternalInput").ap(),
    }
    outs = {"out": nc.dram_tensor("out", [S, DQ], F32, kind="ExternalOutput").ap()}
    with tile.TileContext(nc) as tc:
        with ExitStack() as ctx:
            build_kernel_body(ctx, tc, outs, ins)
    nc.compile()
    _NC_CACHE = nc
    return nc


def make_in_maps(hidden_states, position_ids, q_w, k_w, v_w, o_w):
    import ml_dtypes

    x = np.asarray(hidden_states, dtype=np.float32).reshape(S, HIDDEN)
    xT = np.ascontiguousarray(x.T)
    pos = np.asarray(position_ids).reshape(S).astype(np.float64)
    inv = 1.0 / (THETA ** (np.arange(0, HD, 2, dtype=np.float64) / HD))
    fr = inv[:, None] * pos[None, :]                       # [64, S]
    cos_t = np.cos(fr).astype(np.float32)
    sin_t = np.sin(fr).astype(np.float32)
    u = np.arange(896, dtype=np.int64)[None, :]
    kvi = np.arange(128, dtype=np.int64)[:, None]
    stair = ((u - kvi) >= 384).astype(np.float32)          # [128, 896]

    q_w = np.asarray(q_w, dtype=np.float32)
    k_w = np.asarray(k_w, dtype=np.float32)
    v_w = np.asarray(v_w, dtype=np.float32)
    o_w = np.asarray(o_w, dtype=np.float32)

    in_maps = []
    for c in range(NCORES):
        wqkv = np.ascontiguousarray(np.concatenate(
            [q_w[:, c * DQ:(c + 1) * DQ],
             k_w[:, c * HD:(c + 1) * HD],
             v_w[:, c * HD:(c + 1) * HD]], axis=1))
        owc = np.ascontiguousarray(
            o_w[:, c * DQ:(c + 1) * DQ].astype(ml_dtypes.bfloat16))
        in_maps.append({"xT": xT, "wqkv": wqkv, "ow": owc,
                        "cos_t": cos_t, "sin_t": sin_t, "stair": stair})
    return in_maps


def run(inputs: dict, trace: bool = False):
    """Run on the 8 NeuronCores; returns (full_output, BassKernelResults)."""
    nc = build_program()
    in_maps = make_in_maps(**inputs)
    res = run_bass_kernel_spmd(nc, in_maps, core_ids=list(range(NCORES)),
                               trace=trace)
    full = np.concatenate([res.results[c]["out"] for c in range(NCORES)], axis=1)
    return full.reshape(1, S, HIDDEN), res


def kernel(**inputs) -> np.ndarray:
    out, _ = run(inputs)
    return out


# revision 23
# speedup vs baseline: 1.2918x; 1.0725x over previous
# Mistral sliding-window attention (B=1, S=2048, H=4096, 32 q heads / 8 kv
# heads, window 4096 -> plain causal at this S) on 8 Trainium2 NeuronCores.
#
# Sharding: tensor-parallel over heads. Core c owns q heads 4c..4c+3 and kv
# head c. hidden_states is replicated (transposed on host to [H, S] so the
# contraction dim is the partition dim). Each core computes its attention
# output slice, per-head AllGathers (bf16) assemble the full attention while
# later heads still compute, and each core computes a 512-column slice of
# o_proj; the host concatenates the 8 column slices.
#
# v2 notes:
# - QKV path stays fp32 (float32r matmuls run full-rate at moving dim 512);
#   the o_proj path (attention outs -> AllGather -> gather reads -> o_w) is
#   bf16, halving collective+DMA bytes there at ~0.4% quantization error.
# - Weight tiles 4..31 stream on the scalar engine's DMA queue, everything
#   else on the sync queue: chunk 0 needs ~20MB inside its ~45us of
#   matmuls, more than one queue's bandwidth.
# - qT/kT/V live in per-chunk tiles: tile-granularity dependency tracking
#   would otherwise make the first attention matmul wait for the LAST
#   chunk's rope writes.
# - Attention is one flat software-pipelined stream over (head, chunk,
#   j-pair) with 2 pairs of lookahead, so the PE crosses chunk/head
#   boundaries without waiting for the exp -> mask chain of the last pair.
# - attn_loc writes go out on the scalar engine's DMA queue so the
#   per-head AllGather trigger fires promptly instead of behind the
#   sync queue's bulk traffic.

from contextlib import ExitStack

import numpy as np

import concourse.bacc as bacc
import concourse.bass as bass
import concourse.mybir as mybir
import concourse.tile as tile
from concourse.bass_utils import run_bass_kernel_spmd
from concourse.masks import make_identity

HIDDEN = 4096
NH = 32
NKV = 8
HD = 128
THETA = 10000.0
S = 2048
NCORES = 8

QH = NH // NCORES          # 4 q heads per core
DQ = QH * HD               # 512 (per-core q/attn width)
DOUT = DQ + 2 * HD         # 768 = q heads + k + v projection width
MT = DOUT // 128           # 6 projection m-tiles (0..3 q, 4 k, 5 v)
KT = HIDDEN // 128         # 32 contraction tiles
KG = 4                     # x-load group: k-tiles per DMA
TCH = 512                  # token chunk (matmul moving dim)
NTCH = S // TCH            # 4
KVT = S // 128             # 16 kv tiles
SCALE = 1.0 / float(np.sqrt(HD))

F32 = mybir.dt.float32
F32R = mybir.dt.float32r
BF16 = mybir.dt.bfloat16
EXP = mybir.ActivationFunctionType.Exp


def _rope(nc, pool, src, dst, cs, sn):
    """dst = src*cos + rotate_half(src)*sin, in [d, tok] layout.

    src/dst are [128, n]; cs/sn are [64, n] (the two 64-row halves share
    frequencies). rotate_half: rows 0:64 get -src[64:128], rows 64:128 get
    src[0:64].
    """
    top, bot = src[0:64, :], src[64:128, :]
    ta = pool.tile([64, TCH], F32, name="rope_a")
    tb = pool.tile([64, TCH], F32, name="rope_b")
    nc.vector.tensor_mul(ta, top, cs)
    nc.vector.tensor_mul(tb, bot, sn)
    nc.vector.tensor_sub(dst[0:64, :], ta, tb)
    nc.vector.tensor_mul(ta, bot, cs)
    nc.vector.tensor_mul(tb, top, sn)
    nc.vector.tensor_add(dst[64:128, :], ta, tb)


def build_kernel_body(ctx: ExitStack, tc: tile.TileContext, outs, ins):
    nc = tc.nc
    xT, wqkv, ow, cos_t, sin_t, stair = (
        ins["xT"], ins["wqkv"], ins["ow"], ins["cos_t"], ins["sin_t"],
        ins["stair"],
    )
    out = outs["out"]

    # per-head bounce + gather buffers so each head's AllGather can fire as
    # soon as that head's attention is done (overlaps comm with compute)
    attn_loc = [nc.dram_tensor(f"attn_loc{h}", [HD, S], BF16).ap()
                for h in range(QH)]
    attn_gat = [nc.dram_tensor(f"attn_gat{h}", [NCORES * HD, S], BF16,
                               addr_space="Shared").ap()
                for h in range(QH)]

    singles = ctx.enter_context(tc.tile_pool(name="singles", bufs=1))
    stair_sb = singles.tile([128, 896], F32)
    ones_sb = singles.tile([128, 128], F32R)

    # per-chunk projection outputs, [d, tok] layout (separate tiles per
    # chunk so attention's dependencies stay chunk-granular)
    qTt = [singles.tile([128, QH, TCH], F32R, name=f"qT{t}")
           for t in range(NTCH)]
    kTt = [singles.tile([128, TCH], F32R, name=f"kT{t}")
           for t in range(NTCH)]
    Vt = [singles.tile([128, 4, HD], F32R, name=f"V{t}")
          for t in range(NTCH)]

    def kslice(j):  # kv tile j of roped K, [128 d, 128 tok]
        return kTt[j // 4][:, (j % 4) * 128:(j % 4 + 1) * 128]

    def vtile(j):   # kv tile j of V, [128 tok, 128 d]
        return Vt[j // 4][:, j % 4, :]

    # ---- phase 1: QKV projection + RoPE --------------------------------
    with (
        tc.tile_pool(name="wq", bufs=1) as wp,
        tc.tile_pool(name="xt", bufs=3) as xp,
        tc.tile_pool(name="qk", bufs=2) as qkp,
        tc.tile_pool(name="rope", bufs=2) as rp,
        tc.tile_pool(name="p1ps", bufs=1, space="PSUM") as pp1,
    ):
        cos_sb = wp.tile([64, S], F32)
        sin_sb = wp.tile([64, S], F32)
        ident_sb = wp.tile([128, 128], F32)
        ones_f = wp.tile([128, 128], F32)

        wq3 = wqkv.rearrange("(k p) d -> p k d", p=128)
        x3 = xT.rearrange("(k p) s -> p k s", p=128)
        # first x chunk + first weight tiles on the sync queue so the PE
        # starts immediately; the remaining weight tiles stream on the
        # scalar engine's DMA queue in parallel
        w_sb = [wp.tile([128, DOUT], BF16, name=f"w{k}", tag=f"w{k}")
                for k in range(KT)]
        xg0 = xp.tile([128, KG, TCH], BF16, name="xg")
        nc.sync.dma_start(out=xg0, in_=x3[:, 0:KG, 0:TCH])
        for k in range(KT):
            nc.scalar.dma_start(out=w_sb[k], in_=wq3[:, k, :])
        nc.sync.dma_start(out=cos_sb, in_=cos_t)
        nc.sync.dma_start(out=sin_sb, in_=sin_t)
        nc.sync.dma_start(out=stair_sb, in_=stair)
        nc.vector.memset(ones_f, 1.0)
        nc.vector.tensor_copy(ones_sb, ones_f)
        make_identity(nc, ident_sb)

        for t in range(NTCH):
            ps = [pp1.tile([128, TCH], F32, name=f"p1_{m}", tag=f"p1_{m}")
                  for m in range(MT)]
            for kg in range(KT // KG):
                if t == 0 and kg == 0:
                    xg = xg0
                else:
                    xg = xp.tile([128, KG, TCH], BF16, name="xg")
                    nc.sync.dma_start(
                        out=xg,
                        in_=x3[:, kg * KG:(kg + 1) * KG, t * TCH:(t + 1) * TCH])
                for ki in range(KG):
                    k = kg * KG + ki
                    for m in range(MT):
                        nc.tensor.matmul(
                            ps[m],
                            lhsT=w_sb[k][:, m * 128:(m + 1) * 128],
                            rhs=xg[:, ki, :],
                            start=(k == 0), stop=(k == KT - 1),
                        )
            # ACT copies the V psum -> SBUF; PE transposes it to [tok, d]
            vT = qkp.tile([128, TCH], F32, name="vT")
            nc.scalar.copy(out=vT, in_=ps[QH + 1])
            for jj in range(4):
                pv = pp1.tile([128, 128], F32, name="pvt", tag="pvt")
                nc.tensor.transpose(pv, vT[:, jj * 128:(jj + 1) * 128],
                                    ident_sb)
                nc.scalar.copy(out=Vt[t][:, jj, :], in_=pv)
            cs = cos_sb[:, t * TCH:(t + 1) * TCH]
            sn = sin_sb[:, t * TCH:(t + 1) * TCH]
            for h in range(QH):
                _rope(nc, rp, ps[h], qTt[t][:, h, :], cs, sn)
            _rope(nc, rp, ps[QH], kTt[t], cs, sn)

    # ---- phase 2: attention, per-head AllGather ------------------------
    # ---- phase 3: o_proj on gathered bf16 attention --------------------
    with (
        tc.tile_pool(name="pt", bufs=4) as ptp,
        tc.tile_pool(name="ao", bufs=2) as aop,
        tc.tile_pool(name="ow", bufs=16) as owp,
        tc.tile_pool(name="at", bufs=2) as atp,
        tc.tile_pool(name="acc", bufs=1) as accp,
        tc.tile_pool(name="p2sc", bufs=2, space="PSUM") as pp2,
        tc.tile_pool(name="p2po", bufs=2, space="PSUM") as pop,
        tc.tile_pool(name="p2ps", bufs=2, space="PSUM") as psp,
    ):
        # o_proj output accumulator: acc[:, b, :] = out rows b*128:(b+1)*128
        acc = accp.tile([128, S // 128, TCH], F32)
        ow3 = ow.rearrange("(k p) d -> p k d", p=128)
        ag3 = [attn_gat[h].rearrange("(r p) s -> p r s", p=128)
               for h in range(QH)]
        out3 = out.rearrange("(b p) d -> p b d", p=128)

        def issue_qk(h, c, p):
            sc = pp2.tile([128, 2, TCH], F32, name="sc", tag="sc")
            qslice = qTt[c][:, h, :]
            for i in range(2):
                j = 2 * p + i
                nc.tensor.matmul(sc[:, i, :], lhsT=kslice(j), rhs=qslice,
                                 start=True, stop=True)
            pt = ptp.tile([128, 2, TCH], F32R, name="pt", tag="pt")
            nc.scalar.activation(pt, sc, EXP, scale=SCALE)
            for i in range(2):
                j = 2 * p + i
                rdiag = j - 4 * c
                if rdiag >= 0:  # tile touches the causal diagonal
                    off = 384 - rdiag * 128
                    nc.vector.tensor_mul(pt[:, i, :], pt[:, i, :],
                                         stair_sb[:, off:off + TCH])
            return pt

        def issue_pv(st, p, pt):
            jmax = 4 * st["c"] + 3
            for i in range(2):
                j = 2 * p + i
                nc.tensor.matmul(st["po"], lhsT=vtile(j), rhs=pt[:, i, :],
                                 start=(j == 0), stop=(j == jmax))
                nc.tensor.matmul(st["ps"], lhsT=ones_sb, rhs=pt[:, i, :],
                                 start=(j == 0), stop=(j == jmax))

        def finalize_chunk(st):
            h, c = st["h"], st["c"]
            rec = aop.tile([128, TCH], F32, name="rec")
            nc.vector.reciprocal(rec, st["ps"])
            ao = aop.tile([128, TCH], BF16, name="ao")
            nc.vector.tensor_mul(ao, st["po"], rec)
            # scalar-engine DMA queue: keeps the AllGather's input writes
            # off the congested sync queue so the gather triggers promptly
            nc.scalar.dma_start(
                out=attn_loc[h][:, c * TCH:(c + 1) * TCH], in_=ao)

        def allgather_head(h):
            nc.gpsimd.collective_compute(
                "AllGather",
                mybir.AluOpType.bypass,
                ins=[attn_loc[h][:, :]],
                outs=[attn_gat[h][:, :]],
                replica_groups=[list(range(NCORES))],
            )

        # one flat software-pipelined stream over (head, chunk, j-pair):
        # PV lags QK by LOOK pairs so the PE never waits on exp/mask
        LOOK = 2
        seq = []
        for h in range(QH):
            for c in range(NTCH):
                st = {"h": h, "c": c, "po": None, "ps": None}
                for p in range(2 * (c + 1)):
                    seq.append((st, p))
        pending = []

        def pump(lim):
            while len(pending) > lim:
                st, p, pt = pending.pop(0)
                issue_pv(st, p, pt)
                if 2 * p + 1 == 4 * st["c"] + 3:  # last pair of the chunk
                    finalize_chunk(st)
                    if st["c"] == NTCH - 1:
                        allgather_head(st["h"])

        for st, p in seq:
            if st["po"] is None:
                st["po"] = pop.tile([128, TCH], F32, name="po", tag="po")
                st["ps"] = psp.tile([128, TCH], F32, name="ps", tag="ps")
            pt = issue_qk(st["h"], st["c"], p)
            pending.append((st, p, pt))
            pump(LOOK)
        pump(0)

        def oproj_load_weights(h):
            ows = []
            for r in range(NCORES):
                owk = owp.tile([128, DQ], BF16, name="owk", tag="owk")
                nc.sync.dma_start(out=owk, in_=ow3[:, r * QH + h, :])
                ows.append(owk)
            return ows

        def oproj_chunk(h, g, ows):
            """acc[:, 4g:4g+4, :] += sum_r at(r, h) @ ow(r, h) for 512 toks."""
            at = atp.tile([128, NCORES, TCH], BF16, name="at", tag="at")
            nc.sync.dma_start(
                out=at, in_=ag3[h][:, :, g * TCH:(g + 1) * TCH])
            for mp in range(2):
                pcs = pp2.tile([128, 2, TCH], F32, name="pc", tag="sc")
                for r in range(NCORES):
                    for i, mi in enumerate((2 * mp, 2 * mp + 1)):
                        nc.tensor.matmul(
                            pcs[:, i, :],
                            lhsT=at[:, r, mi * 128:(mi + 1) * 128],
                            rhs=ows[r],
                            start=(r == 0), stop=(r == NCORES - 1),
                        )
                for i, mi in enumerate((2 * mp, 2 * mp + 1)):
                    b = g * 4 + mi
                    if h == 0:
                        nc.scalar.copy(out=acc[:, b, :], in_=pcs[:, i, :])
                    else:
                        nc.vector.tensor_add(acc[:, b, :], acc[:, b, :],
                                             pcs[:, i, :])
            if h == QH - 1:
                nc.sync.dma_start(out=out3[:, 4 * g:4 * g + 4, :],
                                  in_=acc[:, 4 * g:4 * g + 4, :])

        for h in range(QH):
            ows = oproj_load_weights(h)
            for g in range(NTCH):
                oproj_chunk(h, g, ows)


_NC_CACHE = None


def build_program():
    global _NC_CACHE
    if _NC_CACHE is not None:
        return _NC_CACHE
    nc = bacc.Bacc("TRN2", target_bir_lowering=False, debug=False,
                   num_devices=NCORES)
    ins = {
        "xT": nc.dram_tensor("xT", [HIDDEN, S], BF16, kind="ExternalInput").ap(),
        "wqkv": nc.dram_tensor("wqkv", [HIDDEN, DOUT], BF16,
                               kind="ExternalInput").ap(),
        "ow": nc.dram_tensor("ow", [HIDDEN, DQ], BF16, kind="ExternalInput").ap(),
        "cos_t": nc.dram_tensor("cos_t", [64, S], F32, kind="ExternalInput").ap(),
        "sin_t": nc.dram_tensor("sin_t", [64, S], F32, kind="ExternalInput").ap(),
        "stair": nc.dram_tensor("stair", [128, 896], F32,
                                kind="ExternalInput").ap(),
    }
    outs = {"out": nc.dram_tensor("out", [S, DQ], F32, kind="ExternalOutput").ap()}
    with tile.TileContext(nc) as tc:
        with ExitStack() as ctx:
            build_kernel_body(ctx, tc, outs, ins)
    nc.compile()
    _NC_CACHE = nc
    return nc


def make_in_maps(hidden_states, position_ids, q_w, k_w, v_w, o_w):
    import ml_dtypes

    x = np.asarray(hidden_states, dtype=np.float32).reshape(S, HIDDEN)
    xT = np.ascontiguousarray(x.T.astype(ml_dtypes.bfloat16))
    pos = np.asarray(position_ids).reshape(S).astype(np.float64)
    inv = 1.0 / (THETA ** (np.arange(0, HD, 2, dtype=np.float64) / HD))
    fr = inv[:, None] * pos[None, :]                       # [64, S]
    cos_t = np.cos(fr).astype(np.float32)
    sin_t = np.sin(fr).astype(np.float32)
    u = np.arange(896, dtype=np.int64)[None, :]
    kvi = np.arange(128, dtype=np.int64)[:, None]
    stair = ((u - kvi) >= 384).astype(np.float32)          # [128, 896]

    q_w = np.asarray(q_w, dtype=np.float32)
    k_w = np.asarray(k_w, dtype=np.float32)
    v_w = np.asarray(v_w, dtype=np.float32)
    o_w = np.asarray(o_w, dtype=np.float32)

    in_maps = []
    for c in range(NCORES):
        wqkv = np.ascontiguousarray(np.concatenate(
            [q_w[:, c * DQ:(c + 1) * DQ],
             k_w[:, c * HD:(c + 1) * HD],
             v_w[:, c * HD:(c + 1) * HD]], axis=1).astype(ml_dtypes.bfloat16))
        owc = np.ascontiguousarray(
            o_w[:, c * DQ:(c + 1) * DQ].astype(ml_dtypes.bfloat16))
        in_maps.append({"xT": xT, "wqkv": wqkv, "ow": owc,
                        "cos_t": cos_t, "sin_t": sin_t, "stair": stair})
    return in_maps


def run(inputs: dict, trace: bool = False):
    """Run on the 8 NeuronCores; returns (full_output, BassKernelResults)."""
    nc = build_program()
    in_maps = make_in_maps(**inputs)
    res = run_bass_kernel_spmd(nc, in_maps, core_ids=list(range(NCORES)),
                               trace=trace)
    full = np.concatenate([res.results[c]["out"] for c in range(NCORES)], axis=1)
    return full.reshape(1, S, HIDDEN), res


def kernel(**inputs) -> np.ndarray:
    out, _ = run(inputs)
    return out


# revision 29
# speedup vs baseline: 1.3461x; 1.0420x over previous
# Mistral sliding-window attention (B=1, S=2048, H=4096, 32 q heads / 8 kv
# heads, window 4096 -> plain causal at this S) on 8 Trainium2 NeuronCores.
#
# Sharding: tensor-parallel over heads. Core c owns q heads 4c..4c+3 and kv
# head c. hidden_states is replicated (transposed on host to [H, S] so the
# contraction dim is the partition dim). Each core computes its attention
# output slice, per-head AllGathers (bf16) assemble the full attention while
# later heads still compute, and each core computes a 512-column slice of
# o_proj; the host concatenates the 8 column slices.
#
# v2 notes:
# - QKV path stays fp32 (float32r matmuls run full-rate at moving dim 512);
#   the o_proj path (attention outs -> AllGather -> gather reads -> o_w) is
#   bf16, halving collective+DMA bytes there at ~0.4% quantization error.
# - Weight tiles 4..31 stream on the scalar engine's DMA queue, everything
#   else on the sync queue: chunk 0 needs ~20MB inside its ~45us of
#   matmuls, more than one queue's bandwidth.
# - qT/kT/V live in per-chunk tiles: tile-granularity dependency tracking
#   would otherwise make the first attention matmul wait for the LAST
#   chunk's rope writes.
# - Attention is one flat software-pipelined stream over (head, chunk,
#   j-pair) with 2 pairs of lookahead, so the PE crosses chunk/head
#   boundaries without waiting for the exp -> mask chain of the last pair.
# - attn_loc writes go out on the scalar engine's DMA queue so the
#   per-head AllGather trigger fires promptly instead of behind the
#   sync queue's bulk traffic.

from contextlib import ExitStack

import numpy as np

import concourse.bacc as bacc
import concourse.bass as bass
import concourse.mybir as mybir
import concourse.tile as tile
from concourse.bass_utils import run_bass_kernel_spmd
from concourse.masks import make_identity

HIDDEN = 4096
NH = 32
NKV = 8
HD = 128
THETA = 10000.0
S = 2048
NCORES = 8

QH = NH // NCORES          # 4 q heads per core
DQ = QH * HD               # 512 (per-core q/attn width)
DOUT = DQ + 2 * HD         # 768 = q heads + k + v projection width
MT = DOUT // 128           # 6 projection m-tiles (0..3 q, 4 k, 5 v)
KT = HIDDEN // 128         # 32 contraction tiles
KG = 4                     # x-load group: k-tiles per DMA
TCH = 512                  # token chunk (matmul moving dim)
NTCH = S // TCH            # 4
KVT = S // 128             # 16 kv tiles
SCALE = 1.0 / float(np.sqrt(HD))
EXP_SHIFT = -9.0   # keep exp() under fp16 max; cancels in the po/den ratio

F32 = mybir.dt.float32
F32R = mybir.dt.float32r
BF16 = mybir.dt.bfloat16
FP16 = mybir.dt.float16
EXP = mybir.ActivationFunctionType.Exp


def _rope(nc, pool, src, dst, cs, sn):
    """dst = src*cos + rotate_half(src)*sin, in [d, tok] layout.

    src/dst are [128, n]; cs/sn are [128, n] with the 64-row frequency
    block duplicated across both halves. rotate_half is materialized with
    single-input copies (cross-partition-base two-input SBUF ops are
    rejected by the BIR verifier), then everything is 128-partition
    aligned.
    """
    rot = pool.tile([128, TCH], F32, name="rope_rot")
    nc.vector.tensor_scalar_mul(rot[0:64, :], src[64:128, :], -1.0)
    nc.vector.tensor_copy(rot[64:128, :], src[0:64, :])
    t1 = pool.tile([128, TCH], F32, name="rope_t1")
    nc.vector.tensor_mul(t1, src, cs)
    nc.vector.tensor_mul(rot, rot, sn)
    nc.vector.tensor_add(dst, t1, rot)


def build_kernel_body(ctx: ExitStack, tc: tile.TileContext, outs, ins):
    nc = tc.nc
    xT, wqkv, ow, cos_t, sin_t, stair = (
        ins["xT"], ins["wqkv"], ins["ow"], ins["cos_t"], ins["sin_t"],
        ins["stair"],
    )
    out = outs["out"]

    # per-head bounce + gather buffers so each head's AllGather can fire as
    # soon as that head's attention is done (overlaps comm with compute)
    attn_loc = [nc.dram_tensor(f"attn_loc{h}", [HD, S], BF16).ap()
                for h in range(QH)]
    attn_gat = [nc.dram_tensor(f"attn_gat{h}", [NCORES * HD, S], BF16,
                               addr_space="Shared").ap()
                for h in range(QH)]

    singles = ctx.enter_context(tc.tile_pool(name="singles", bufs=1))
    stair_sb = singles.tile([128, 896], FP16)
    shift_sb = singles.tile([128, 1], F32)
    ones_sb = singles.tile([128, 128], FP16)

    # per-chunk projection outputs, [d, tok] layout (separate tiles per
    # chunk so attention's dependencies stay chunk-granular)
    qTt = [singles.tile([128, QH, TCH], F32R, name=f"qT{t}")
           for t in range(NTCH)]
    kTt = [singles.tile([128, TCH], F32R, name=f"kT{t}")
           for t in range(NTCH)]
    Vt = [singles.tile([128, 4, HD], FP16, name=f"V{t}")
          for t in range(NTCH)]

    def kslice(j):  # kv tile j of roped K, [128 d, 128 tok]
        return kTt[j // 4][:, (j % 4) * 128:(j % 4 + 1) * 128]

    def vtile(j):   # kv tile j of V, [128 tok, 128 d]
        return Vt[j // 4][:, j % 4, :]

    # ---- phase 1: QKV projection + RoPE --------------------------------
    with (
        tc.tile_pool(name="wq", bufs=1) as wp,
        tc.tile_pool(name="xt", bufs=4) as xp,
        tc.tile_pool(name="qk", bufs=2) as qkp,
        tc.tile_pool(name="rope", bufs=2) as rp,
        tc.tile_pool(name="p1ps", bufs=1, space="PSUM") as pp1,
    ):
        cos_sb = wp.tile([128, S], F32)
        sin_sb = wp.tile([128, S], F32)
        ident_sb = wp.tile([128, 128], F32)
        ones_f = wp.tile([128, 128], F32)

        wq3 = wqkv.rearrange("(k p) d -> p k d", p=128)
        x3 = xT.rearrange("(k p) s -> p k s", p=128)
        # first x chunk + first weight tiles on the sync queue so the PE
        # starts immediately; the remaining weight tiles stream on the
        # scalar engine's DMA queue in parallel
        w_sb = [wp.tile([128, DOUT], BF16, name=f"w{k}", tag=f"w{k}")
                for k in range(KT)]
        xg0 = xp.tile([128, KG, TCH], BF16, name="xg")
        nc.sync.dma_start(out=xg0, in_=x3[:, 0:KG, 0:TCH])
        for k in range(KT):
            nc.scalar.dma_start(out=w_sb[k], in_=wq3[:, k, :])
        nc.sync.dma_start(out=cos_sb, in_=cos_t)
        nc.sync.dma_start(out=sin_sb, in_=sin_t)
        nc.sync.dma_start(out=stair_sb, in_=stair)
        nc.vector.memset(shift_sb, EXP_SHIFT)
        nc.vector.memset(ones_f, 1.0)
        nc.vector.tensor_copy(ones_sb, ones_f)
        make_identity(nc, ident_sb)

        for t in range(NTCH):
            ps = [pp1.tile([128, TCH], F32, name=f"p1_{m}", tag=f"p1_{m}")
                  for m in range(MT)]
            for kg in range(KT // KG):
                if t == 0 and kg == 0:
                    xg = xg0
                else:
                    xg = xp.tile([128, KG, TCH], BF16, name="xg")
                    nc.sync.dma_start(
                        out=xg,
                        in_=x3[:, kg * KG:(kg + 1) * KG, t * TCH:(t + 1) * TCH])
                for ki in range(KG):
                    k = kg * KG + ki
                    for m in range(MT):
                        nc.tensor.matmul(
                            ps[m],
                            lhsT=w_sb[k][:, m * 128:(m + 1) * 128],
                            rhs=xg[:, ki, :],
                            start=(k == 0), stop=(k == KT - 1),
                        )
            # ACT copies psum -> SBUF so the psum banks (and pool) free
            # right after the chunk's matmuls; rope + transposes read copies
            vT = qkp.tile([128, TCH], F32, name="vT")
            nc.scalar.copy(out=vT, in_=ps[QH + 1])
            qk = qkp.tile([128, QH + 1, TCH], F32, name="qk")
            for m in range(QH + 1):
                nc.scalar.copy(out=qk[:, m, :], in_=ps[m])
            for jj in range(4):
                pv = pp1.tile([128, 128], F32, name="pvt", tag="pvt")
                nc.tensor.transpose(pv, vT[:, jj * 128:(jj + 1) * 128],
                                    ident_sb)
                nc.scalar.copy(out=Vt[t][:, jj, :], in_=pv)
            cs = cos_sb[:, t * TCH:(t + 1) * TCH]
            sn = sin_sb[:, t * TCH:(t + 1) * TCH]
            for h in range(QH):
                _rope(nc, rp, qk[:, h, :], qTt[t][:, h, :], cs, sn)
            _rope(nc, rp, qk[:, QH, :], kTt[t], cs, sn)

    # ---- phase 2: attention, per-head AllGather ------------------------
    # ---- phase 3: o_proj on gathered bf16 attention --------------------
    with (
        tc.tile_pool(name="pt", bufs=4) as ptp,
        tc.tile_pool(name="sm", bufs=2) as smp,
        tc.tile_pool(name="ao", bufs=2) as aop,
        tc.tile_pool(name="ow", bufs=16) as owp,
        tc.tile_pool(name="at", bufs=2) as atp,
        tc.tile_pool(name="acc", bufs=1) as accp,
        tc.tile_pool(name="p2sc", bufs=2, space="PSUM") as pp2,
        tc.tile_pool(name="p2po", bufs=2, space="PSUM") as pop,
        tc.tile_pool(name="p2ps", bufs=2, space="PSUM") as psp,
    ):
        # o_proj output accumulator: acc[:, b, :] = out rows b*128:(b+1)*128
        acc = accp.tile([128, S // 128, TCH], F32)
        ow3 = ow.rearrange("(k p) d -> p k d", p=128)
        ag3 = [attn_gat[h].rearrange("(r p) s -> p r s", p=128)
               for h in range(QH)]
        out3 = out.rearrange("(b p) d -> p b d", p=128)

        def issue_qk(h, c, p):
            sc = pp2.tile([128, 2, TCH], F32, name="sc", tag="sc")
            qslice = qTt[c][:, h, :]
            for i in range(2):
                j = 2 * p + i
                nc.tensor.matmul(sc[:, i, :], lhsT=kslice(j), rhs=qslice,
                                 start=True, stop=True)
            pt = ptp.tile([128, 2, TCH], FP16, name="pt", tag="pt")
            nc.scalar.activation(pt, sc, EXP, scale=SCALE, bias=shift_sb)
            for i in range(2):
                j = 2 * p + i
                rdiag = j - 4 * c
                if rdiag >= 0:  # tile touches the causal diagonal
                    off = 384 - rdiag * 128
                    nc.vector.tensor_mul(pt[:, i, :], pt[:, i, :],
                                         stair_sb[:, off:off + TCH])
            return pt

        def issue_pv(st, p, pt):
            jmax = 4 * st["c"] + 3
            for i in range(2):
                j = 2 * p + i
                nc.tensor.matmul(st["po"], lhsT=vtile(j), rhs=pt[:, i, :],
                                 start=(j == 0), stop=(j == jmax))

        def accumulate(st, p, pt):
            # fp16 softmax-denominator partials on the DVE (2x 16-bit rate);
            # the per-chunk kv-sum happens in one ones-matmul at finalize
            if st["sum"] is None:
                st["sum"] = smp.tile([128, TCH], FP16, name="sm")
                nc.vector.tensor_add(st["sum"], pt[:, 0, :], pt[:, 1, :])
            else:
                nc.vector.tensor_add(st["sum"], st["sum"], pt[:, 0, :])
                nc.vector.tensor_add(st["sum"], st["sum"], pt[:, 1, :])

        def finalize_chunk(st):
            h, c = st["h"], st["c"]
            nc.tensor.matmul(st["ps"], lhsT=ones_sb, rhs=st["sum"],
                             start=True, stop=True)
            rec = aop.tile([128, TCH], F32, name="rec")
            nc.vector.reciprocal(rec, st["ps"])
            ao = aop.tile([128, TCH], BF16, name="ao")
            nc.vector.tensor_mul(ao, st["po"], rec)
            # scalar-engine DMA queue: keeps the AllGather's input writes
            # off the congested sync queue so the gather triggers promptly
            nc.scalar.dma_start(
                out=attn_loc[h][:, c * TCH:(c + 1) * TCH], in_=ao)

        def allgather_head(h):
            nc.gpsimd.collective_compute(
                "AllGather",
                mybir.AluOpType.bypass,
                ins=[attn_loc[h][:, :]],
                outs=[attn_gat[h][:, :]],
                replica_groups=[list(range(NCORES))],
            )

        # one flat software-pipelined stream over (head, chunk, j-pair):
        # PV lags QK by LOOK pairs so the PE never waits on exp/mask
        LOOK = 3
        seq = []
        for h in range(QH):
            for c in range(NTCH):
                st = {"h": h, "c": c, "po": None, "ps": None,
                      "sum": None}
                for p in range(2 * (c + 1)):
                    seq.append((st, p))
        pending = []

        def pump(lim):
            while len(pending) > lim:
                st, p, pt = pending.pop(0)
                issue_pv(st, p, pt)
                if 2 * p + 1 == 4 * st["c"] + 3:  # last pair of the chunk
                    finalize_chunk(st)
                    if st["c"] == NTCH - 1:
                        allgather_head(st["h"])

        for st, p in seq:
            if st["po"] is None:
                st["po"] = pop.tile([128, TCH], F32, name="po", tag="po")
                st["ps"] = psp.tile([128, TCH], F32, name="ps", tag="ps")
            pt = issue_qk(st["h"], st["c"], p)
            accumulate(st, p, pt)
            pending.append((st, p, pt))
            pump(LOOK)
        pump(0)

        def oproj_load_weights(h):
            ows = []
            for r in range(NCORES):
                owk = owp.tile([128, DQ], BF16, name="owk", tag="owk")
                nc.sync.dma_start(out=owk, in_=ow3[:, r * QH + h, :])
                ows.append(owk)
            return ows

        def oproj_chunk(h, g, ows):
            """acc[:, 4g:4g+4, :] += sum_r at(r, h) @ ow(r, h) for 512 toks."""
            at = atp.tile([128, NCORES, TCH], BF16, name="at", tag="at")
            nc.sync.dma_start(
                out=at, in_=ag3[h][:, :, g * TCH:(g + 1) * TCH])
            for mp in range(2):
                pcs = pp2.tile([128, 2, TCH], F32, name="pc", tag="sc")
                for r in range(NCORES):
                    for i, mi in enumerate((2 * mp, 2 * mp + 1)):
                        nc.tensor.matmul(
                            pcs[:, i, :],
                            lhsT=at[:, r, mi * 128:(mi + 1) * 128],
                            rhs=ows[r],
                            start=(r == 0), stop=(r == NCORES - 1),
                        )
                for i, mi in enumerate((2 * mp, 2 * mp + 1)):
                    b = g * 4 + mi
                    if h == 0:
                        nc.scalar.copy(out=acc[:, b, :], in_=pcs[:, i, :])
                    else:
                        nc.vector.tensor_add(acc[:, b, :], acc[:, b, :],
                                             pcs[:, i, :])
            if h == QH - 1:
                nc.sync.dma_start(out=out3[:, 4 * g:4 * g + 4, :],
                                  in_=acc[:, 4 * g:4 * g + 4, :])

        for h in range(QH):
            ows = oproj_load_weights(h)
            for g in range(NTCH):
                oproj_chunk(h, g, ows)


_NC_CACHE = None


def build_program():
    global _NC_CACHE
    if _NC_CACHE is not None:
        return _NC_CACHE
    nc = bacc.Bacc("TRN2", target_bir_lowering=False, debug=False,
                   num_devices=NCORES)
    ins = {
        "xT": nc.dram_tensor("xT", [HIDDEN, S], BF16, kind="ExternalInput").ap(),
        "wqkv": nc.dram_tensor("wqkv", [HIDDEN, DOUT], BF16,
                               kind="ExternalInput").ap(),
        "ow": nc.dram_tensor("ow", [HIDDEN, DQ], BF16, kind="ExternalInput").ap(),
        "cos_t": nc.dram_tensor("cos_t", [128, S], F32, kind="ExternalInput").ap(),
        "sin_t": nc.dram_tensor("sin_t", [128, S], F32, kind="ExternalInput").ap(),
        "stair": nc.dram_tensor("stair", [128, 896], FP16,
                                kind="ExternalInput").ap(),
    }
    outs = {"out": nc.dram_tensor("out", [S, DQ], F32, kind="ExternalOutput").ap()}
    with tile.TileContext(nc) as tc:
        with ExitStack() as ctx:
            build_kernel_body(ctx, tc, outs, ins)
    nc.compile()
    _NC_CACHE = nc
    return nc


def make_in_maps(hidden_states, position_ids, q_w, k_w, v_w, o_w):
    import ml_dtypes

    x = np.asarray(hidden_states, dtype=np.float32).reshape(S, HIDDEN)
    xT = np.ascontiguousarray(x.T.astype(ml_dtypes.bfloat16))
    pos = np.asarray(position_ids).reshape(S).astype(np.float64)
    inv = 1.0 / (THETA ** (np.arange(0, HD, 2, dtype=np.float64) / HD))
    fr = inv[:, None] * pos[None, :]                       # [64, S]
    cos_t = np.concatenate([np.cos(fr), np.cos(fr)], axis=0).astype(np.float32)
    sin_t = np.concatenate([np.sin(fr), np.sin(fr)], axis=0).astype(np.float32)
    u = np.arange(896, dtype=np.int64)[None, :]
    kvi = np.arange(128, dtype=np.int64)[:, None]
    stair = ((u - kvi) >= 384).astype(np.float16)          # [128, 896]

    q_w = np.asarray(q_w, dtype=np.float32)
    k_w = np.asarray(k_w, dtype=np.float32)
    v_w = np.asarray(v_w, dtype=np.float32)
    o_w = np.asarray(o_w, dtype=np.float32)

    in_maps = []
    for c in range(NCORES):
        wqkv = np.ascontiguousarray(np.concatenate(
            [q_w[:, c * DQ:(c + 1) * DQ],
             k_w[:, c * HD:(c + 1) * HD],
             v_w[:, c * HD:(c + 1) * HD]], axis=1).astype(ml_dtypes.bfloat16))
        owc = np.ascontiguousarray(
            o_w[:, c * DQ:(c + 1) * DQ].astype(ml_dtypes.bfloat16))
        in_maps.append({"xT": xT, "wqkv": wqkv, "ow": owc,
                        "cos_t": cos_t, "sin_t": sin_t, "stair": stair})
    return in_maps


def run(inputs: dict, trace: bool = False):
    """Run on the 8 NeuronCores; returns (full_output, BassKernelResults)."""
    nc = build_program()
    in_maps = make_in_maps(**inputs)
    res = run_bass_kernel_spmd(nc, in_maps, core_ids=list(range(NCORES)),
                               trace=trace)
    full = np.concatenate([res.results[c]["out"] for c in range(NCORES)], axis=1)
    return full.reshape(1, S, HIDDEN), res


def kernel(**inputs) -> np.ndarray:
    out, _ = run(inputs)
    return out


# revision 31
# speedup vs baseline: 1.4018x; 1.0414x over previous
# Mistral sliding-window attention (B=1, S=2048, H=4096, 32 q heads / 8 kv
# heads, window 4096 -> plain causal at this S) on 8 Trainium2 NeuronCores.
#
# Sharding: tensor-parallel over heads. Core c owns q heads 4c..4c+3 and kv
# head c. hidden_states is replicated (transposed on host to [H, S] so the
# contraction dim is the partition dim). Each core computes its attention
# output slice, per-head AllGathers (bf16) assemble the full attention while
# later heads still compute, and each core computes a 512-column slice of
# o_proj; the host concatenates the 8 column slices.
#
# v2 notes:
# - QKV path stays fp32 (float32r matmuls run full-rate at moving dim 512);
#   the o_proj path (attention outs -> AllGather -> gather reads -> o_w) is
#   bf16, halving collective+DMA bytes there at ~0.4% quantization error.
# - Weight tiles 4..31 stream on the scalar engine's DMA queue, everything
#   else on the sync queue: chunk 0 needs ~20MB inside its ~45us of
#   matmuls, more than one queue's bandwidth.
# - qT/kT/V live in per-chunk tiles: tile-granularity dependency tracking
#   would otherwise make the first attention matmul wait for the LAST
#   chunk's rope writes.
# - Attention is one flat software-pipelined stream over (head, chunk,
#   j-pair) with 2 pairs of lookahead, so the PE crosses chunk/head
#   boundaries without waiting for the exp -> mask chain of the last pair.
# - attn_loc writes go out on the scalar engine's DMA queue so the
#   per-head AllGather trigger fires promptly instead of behind the
#   sync queue's bulk traffic.

from contextlib import ExitStack

import numpy as np

import concourse.bacc as bacc
import concourse.bass as bass
import concourse.mybir as mybir
import concourse.tile as tile
from concourse.bass_utils import run_bass_kernel_spmd
from concourse.masks import make_identity

HIDDEN = 4096
NH = 32
NKV = 8
HD = 128
THETA = 10000.0
S = 2048
NCORES = 8

QH = NH // NCORES          # 4 q heads per core
DQ = QH * HD               # 512 (per-core q/attn width)
DOUT = DQ + 2 * HD         # 768 = q heads + k + v projection width
MT = DOUT // 128           # 6 projection m-tiles (0..3 q, 4 k, 5 v)
KT = HIDDEN // 128         # 32 contraction tiles
KG = 4                     # x-load group: k-tiles per DMA
TCH = 512                  # token chunk (matmul moving dim)
NTCH = S // TCH            # 4
KVT = S // 128             # 16 kv tiles
SCALE = 1.0 / float(np.sqrt(HD))
EXP_SHIFT = -9.0   # keep exp() under fp16 max; cancels in the po/den ratio

F32 = mybir.dt.float32
F32R = mybir.dt.float32r
BF16 = mybir.dt.bfloat16
FP16 = mybir.dt.float16
EXP = mybir.ActivationFunctionType.Exp


def _rope(nc, pool, src, dst, cs, sn):
    """dst = src*cos + rotate_half(src)*sin, in [d, tok] layout.

    src/dst are [128, n]; cs/sn are [128, n] with the 64-row frequency
    block duplicated across both halves. rotate_half is materialized with
    single-input copies (cross-partition-base two-input SBUF ops are
    rejected by the BIR verifier), then everything is 128-partition
    aligned.
    """
    rot = pool.tile([128, TCH], BF16, name="rope_rot")
    nc.vector.tensor_scalar_mul(rot[0:64, :], src[64:128, :], -1.0)
    nc.vector.tensor_copy(rot[64:128, :], src[0:64, :])
    t1 = pool.tile([128, TCH], BF16, name="rope_t1")
    nc.vector.tensor_mul(t1, src, cs)
    nc.vector.tensor_mul(rot, rot, sn)
    nc.vector.tensor_add(dst, t1, rot)


def build_kernel_body(ctx: ExitStack, tc: tile.TileContext, outs, ins):
    nc = tc.nc
    xT, wqkv, ow, cos_t, sin_t, stair = (
        ins["xT"], ins["wqkv"], ins["ow"], ins["cos_t"], ins["sin_t"],
        ins["stair"],
    )
    out = outs["out"]

    # per-head bounce + gather buffers so each head's AllGather can fire as
    # soon as that head's attention is done (overlaps comm with compute)
    attn_loc = [nc.dram_tensor(f"attn_loc{h}", [HD, S], BF16).ap()
                for h in range(QH)]
    attn_gat = [nc.dram_tensor(f"attn_gat{h}", [NCORES * HD, S], BF16,
                               addr_space="Shared").ap()
                for h in range(QH)]

    singles = ctx.enter_context(tc.tile_pool(name="singles", bufs=1))
    stair_sb = singles.tile([128, 896], FP16)
    shift_sb = singles.tile([128, 1], F32)
    ones_sb = singles.tile([128, 128], FP16)

    # per-chunk projection outputs, [d, tok] layout (separate tiles per
    # chunk so attention's dependencies stay chunk-granular)
    qTt = [singles.tile([128, QH, TCH], BF16, name=f"qT{t}")
           for t in range(NTCH)]
    kTt = [singles.tile([128, TCH], BF16, name=f"kT{t}")
           for t in range(NTCH)]
    Vt = [singles.tile([128, 4, HD], FP16, name=f"V{t}")
          for t in range(NTCH)]

    def kslice(j):  # kv tile j of roped K, [128 d, 128 tok]
        return kTt[j // 4][:, (j % 4) * 128:(j % 4 + 1) * 128]

    def vtile(j):   # kv tile j of V, [128 tok, 128 d]
        return Vt[j // 4][:, j % 4, :]

    # ---- phase 1: QKV projection + RoPE --------------------------------
    with (
        tc.tile_pool(name="wq", bufs=1) as wp,
        tc.tile_pool(name="xt", bufs=4) as xp,
        tc.tile_pool(name="qk", bufs=2) as qkp,
        tc.tile_pool(name="rope", bufs=2) as rp,
        tc.tile_pool(name="p1ps", bufs=1, space="PSUM") as pp1,
    ):
        cos_sb = wp.tile([128, S], BF16)
        sin_sb = wp.tile([128, S], BF16)
        ident_sb = wp.tile([128, 128], F32)
        ones_f = wp.tile([128, 128], F32)

        wq3 = wqkv.rearrange("(k p) d -> p k d", p=128)
        x3 = xT.rearrange("(k p) s -> p k s", p=128)
        # first x chunk + first weight tiles on the sync queue so the PE
        # starts immediately; the remaining weight tiles stream on the
        # scalar engine's DMA queue in parallel
        w_sb = [wp.tile([128, DOUT], BF16, name=f"w{k}", tag=f"w{k}")
                for k in range(KT)]
        xg0 = xp.tile([128, KG, TCH], BF16, name="xg")
        nc.sync.dma_start(out=xg0, in_=x3[:, 0:KG, 0:TCH])
        for k in range(KT):
            nc.scalar.dma_start(out=w_sb[k], in_=wq3[:, k, :])
        nc.sync.dma_start(out=cos_sb, in_=cos_t)
        nc.sync.dma_start(out=sin_sb, in_=sin_t)
        nc.sync.dma_start(out=stair_sb, in_=stair)
        nc.vector.memset(shift_sb, EXP_SHIFT)
        nc.vector.memset(ones_f, 1.0)
        nc.vector.tensor_copy(ones_sb, ones_f)
        make_identity(nc, ident_sb)

        for t in range(NTCH):
            ps = [pp1.tile([128, TCH], F32, name=f"p1_{m}", tag=f"p1_{m}")
                  for m in range(MT)]
            for kg in range(KT // KG):
                if t == 0 and kg == 0:
                    xg = xg0
                else:
                    xg = xp.tile([128, KG, TCH], BF16, name="xg")
                    nc.sync.dma_start(
                        out=xg,
                        in_=x3[:, kg * KG:(kg + 1) * KG, t * TCH:(t + 1) * TCH])
                for ki in range(KG):
                    k = kg * KG + ki
                    for m in range(MT):
                        nc.tensor.matmul(
                            ps[m],
                            lhsT=w_sb[k][:, m * 128:(m + 1) * 128],
                            rhs=xg[:, ki, :],
                            start=(k == 0), stop=(k == KT - 1),
                        )
            # ACT copies psum -> SBUF so the psum banks (and pool) free
            # right after the chunk's matmuls; rope + transposes read copies
            vT = qkp.tile([128, TCH], F32, name="vT")
            nc.scalar.copy(out=vT, in_=ps[QH + 1])
            qk = qkp.tile([128, QH + 1, TCH], BF16, name="qk")
            for m in range(QH + 1):
                nc.scalar.copy(out=qk[:, m, :], in_=ps[m])
            for jj in range(4):
                pv = pp1.tile([128, 128], F32, name="pvt", tag="pvt")
                nc.tensor.transpose(pv, vT[:, jj * 128:(jj + 1) * 128],
                                    ident_sb)
                nc.scalar.copy(out=Vt[t][:, jj, :], in_=pv)
            cs = cos_sb[:, t * TCH:(t + 1) * TCH]
            sn = sin_sb[:, t * TCH:(t + 1) * TCH]
            for h in range(QH):
                _rope(nc, rp, qk[:, h, :], qTt[t][:, h, :], cs, sn)
            _rope(nc, rp, qk[:, QH, :], kTt[t], cs, sn)

    # ---- phase 2: attention, per-head AllGather ------------------------
    # ---- phase 3: o_proj on gathered bf16 attention --------------------
    with (
        tc.tile_pool(name="pt", bufs=4) as ptp,
        tc.tile_pool(name="sm", bufs=2) as smp,
        tc.tile_pool(name="ao", bufs=2) as aop,
        tc.tile_pool(name="ow", bufs=16) as owp,
        tc.tile_pool(name="at", bufs=2) as atp,
        tc.tile_pool(name="acc", bufs=1) as accp,
        tc.tile_pool(name="p2sc", bufs=2, space="PSUM") as pp2,
        tc.tile_pool(name="p2po", bufs=2, space="PSUM") as pop,
        tc.tile_pool(name="p2ps", bufs=2, space="PSUM") as psp,
    ):
        # o_proj output accumulator: acc[:, b, :] = out rows b*128:(b+1)*128
        acc = accp.tile([128, S // 128, TCH], F32)
        ow3 = ow.rearrange("(k p) d -> p k d", p=128)
        ag3 = [attn_gat[h].rearrange("(r p) s -> p r s", p=128)
               for h in range(QH)]
        out3 = out.rearrange("(b p) d -> p b d", p=128)

        def issue_qk(h, c, p):
            sc = pp2.tile([128, 2, TCH], F32, name="sc", tag="sc")
            qslice = qTt[c][:, h, :]
            for i in range(2):
                j = 2 * p + i
                nc.tensor.matmul(sc[:, i, :], lhsT=kslice(j), rhs=qslice,
                                 start=True, stop=True)
            pt = ptp.tile([128, 2, TCH], FP16, name="pt", tag="pt")
            nc.scalar.activation(pt, sc, EXP, scale=SCALE, bias=shift_sb)
            for i in range(2):
                j = 2 * p + i
                rdiag = j - 4 * c
                if rdiag >= 0:  # tile touches the causal diagonal
                    off = 384 - rdiag * 128
                    nc.vector.tensor_mul(pt[:, i, :], pt[:, i, :],
                                         stair_sb[:, off:off + TCH])
            return pt

        def issue_pv(st, p, pt):
            jmax = 4 * st["c"] + 3
            for i in range(2):
                j = 2 * p + i
                nc.tensor.matmul(st["po"], lhsT=vtile(j), rhs=pt[:, i, :],
                                 start=(j == 0), stop=(j == jmax))

        def accumulate(st, p, pt):
            # softmax-denominator partials, balanced across engines: 2/3 of
            # the j-pairs sum on the DVE in fp16 (2x 16-bit rate), 1/3 as
            # PE ones-matmuls into the ps psum; finalize folds the DVE sum
            # into ps with one more ones-matmul
            if p % 3 == 2:
                for i in range(2):
                    nc.tensor.matmul(st["ps"], lhsT=ones_sb, rhs=pt[:, i, :],
                                     start=not st["ps_on"] and i == 0,
                                     stop=False)
                st["ps_on"] = True
            elif st["sum"] is None:
                st["sum"] = smp.tile([128, TCH], FP16, name="sm")
                nc.vector.tensor_add(st["sum"], pt[:, 0, :], pt[:, 1, :])
            else:
                nc.vector.tensor_add(st["sum"], st["sum"], pt[:, 0, :])
                nc.vector.tensor_add(st["sum"], st["sum"], pt[:, 1, :])

        def finalize_chunk(st):
            h, c = st["h"], st["c"]
            nc.tensor.matmul(st["ps"], lhsT=ones_sb, rhs=st["sum"],
                             start=not st["ps_on"], stop=True)
            rec = aop.tile([128, TCH], F32, name="rec")
            nc.vector.reciprocal(rec, st["ps"])
            ao = aop.tile([128, TCH], BF16, name="ao")
            nc.vector.tensor_mul(ao, st["po"], rec)
            # scalar-engine DMA queue: keeps the AllGather's input writes
            # off the congested sync queue so the gather triggers promptly
            nc.scalar.dma_start(
                out=attn_loc[h][:, c * TCH:(c + 1) * TCH], in_=ao)

        def allgather_head(h):
            nc.gpsimd.collective_compute(
                "AllGather",
                mybir.AluOpType.bypass,
                ins=[attn_loc[h][:, :]],
                outs=[attn_gat[h][:, :]],
                replica_groups=[list(range(NCORES))],
            )

        # one flat software-pipelined stream over (head, chunk, j-pair):
        # PV lags QK by LOOK pairs, and each chunk's finalize (denominator
        # matmul + reciprocal) lags its last PV by FLAG pairs, so the PE
        # never waits on the exp/mask chain or the DVE queue
        LOOK = 3
        FLAG = 2
        seq = []
        for h in range(QH):
            for c in range(NTCH):
                st = {"h": h, "c": c, "po": None, "ps": None,
                      "sum": None, "ps_on": False}
                for p in range(2 * (c + 1)):
                    seq.append((st, p))
        pending = []
        fin_q = []

        def flush_finalizers(lim):
            while len(fin_q) > lim or (fin_q and fin_q[0][1] >= FLAG):
                fst, _ = fin_q.pop(0)
                finalize_chunk(fst)
                if fst["c"] == NTCH - 1:
                    allgather_head(fst["h"])

        def pump(lim):
            while len(pending) > lim:
                st, p, pt = pending.pop(0)
                issue_pv(st, p, pt)
                for e in fin_q:
                    e[1] += 1
                if 2 * p + 1 == 4 * st["c"] + 3:  # last pair of the chunk
                    fin_q.append([st, 0])
                flush_finalizers(8)

        for st, p in seq:
            if st["po"] is None:
                st["po"] = pop.tile([128, TCH], F32, name="po", tag="po")
                st["ps"] = psp.tile([128, TCH], F32, name="ps", tag="ps")
            pt = issue_qk(st["h"], st["c"], p)
            accumulate(st, p, pt)
            pending.append((st, p, pt))
            pump(LOOK)
        pump(0)
        flush_finalizers(0)

        def oproj_load_weights(h):
            ows = []
            for r in range(NCORES):
                owk = owp.tile([128, DQ], BF16, name="owk", tag="owk")
                nc.sync.dma_start(out=owk, in_=ow3[:, r * QH + h, :])
                ows.append(owk)
            return ows

        def oproj_chunk(h, g, ows):
            """acc[:, 4g:4g+4, :] += sum_r at(r, h) @ ow(r, h) for 512 toks."""
            at = atp.tile([128, NCORES, TCH], BF16, name="at", tag="at")
            nc.sync.dma_start(
                out=at, in_=ag3[h][:, :, g * TCH:(g + 1) * TCH])
            for mp in range(2):
                pcs = pp2.tile([128, 2, TCH], F32, name="pc", tag="sc")
                for r in range(NCORES):
                    for i, mi in enumerate((2 * mp, 2 * mp + 1)):
                        nc.tensor.matmul(
                            pcs[:, i, :],
                            lhsT=at[:, r, mi * 128:(mi + 1) * 128],
                            rhs=ows[r],
                            start=(r == 0), stop=(r == NCORES - 1),
                        )
                for i, mi in enumerate((2 * mp, 2 * mp + 1)):
                    b = g * 4 + mi
                    if h == 0:
                        nc.scalar.copy(out=acc[:, b, :], in_=pcs[:, i, :])
                    else:
                        nc.vector.tensor_add(acc[:, b, :], acc[:, b, :],
                                             pcs[:, i, :])
            if h == QH - 1:
                nc.sync.dma_start(out=out3[:, 4 * g:4 * g + 4, :],
                                  in_=acc[:, 4 * g:4 * g + 4, :])

        for h in range(QH):
            ows = oproj_load_weights(h)
            for g in range(NTCH):
                oproj_chunk(h, g, ows)


_NC_CACHE = None


def build_program():
    global _NC_CACHE
    if _NC_CACHE is not None:
        return _NC_CACHE
    nc = bacc.Bacc("TRN2", target_bir_lowering=False, debug=False,
                   num_devices=NCORES)
    ins = {
        "xT": nc.dram_tensor("xT", [HIDDEN, S], BF16, kind="ExternalInput").ap(),
        "wqkv": nc.dram_tensor("wqkv", [HIDDEN, DOUT], BF16,
                               kind="ExternalInput").ap(),
        "ow": nc.dram_tensor("ow", [HIDDEN, DQ], BF16, kind="ExternalInput").ap(),
        "cos_t": nc.dram_tensor("cos_t", [128, S], BF16,
                                kind="ExternalInput").ap(),
        "sin_t": nc.dram_tensor("sin_t", [128, S], BF16,
                                kind="ExternalInput").ap(),
        "stair": nc.dram_tensor("stair", [128, 896], FP16,
                                kind="ExternalInput").ap(),
    }
    outs = {"out": nc.dram_tensor("out", [S, DQ], F32, kind="ExternalOutput").ap()}
    with tile.TileContext(nc) as tc:
        with ExitStack() as ctx:
            build_kernel_body(ctx, tc, outs, ins)
    nc.compile()
    _NC_CACHE = nc
    return nc


def make_in_maps(hidden_states, position_ids, q_w, k_w, v_w, o_w):
    import ml_dtypes

    x = np.asarray(hidden_states, dtype=np.float32).reshape(S, HIDDEN)
    xT = np.ascontiguousarray(x.T.astype(ml_dtypes.bfloat16))
    pos = np.asarray(position_ids).reshape(S).astype(np.float64)
    inv = 1.0 / (THETA ** (np.arange(0, HD, 2, dtype=np.float64) / HD))
    fr = inv[:, None] * pos[None, :]                       # [64, S]
    cos_t = np.concatenate([np.cos(fr), np.cos(fr)], axis=0).astype(
        ml_dtypes.bfloat16)
    sin_t = np.concatenate([np.sin(fr), np.sin(fr)], axis=0).astype(
        ml_dtypes.bfloat16)
    u = np.arange(896, dtype=np.int64)[None, :]
    kvi = np.arange(128, dtype=np.int64)[:, None]
    stair = ((u - kvi) >= 384).astype(np.float16)          # [128, 896]

    q_w = np.asarray(q_w, dtype=np.float32)
    k_w = np.asarray(k_w, dtype=np.float32)
    v_w = np.asarray(v_w, dtype=np.float32)
    o_w = np.asarray(o_w, dtype=np.float32)

    in_maps = []
    for c in range(NCORES):
        wqkv = np.ascontiguousarray(np.concatenate(
            [q_w[:, c * DQ:(c + 1) * DQ],
             k_w[:, c * HD:(c + 1) * HD],
             v_w[:, c * HD:(c + 1) * HD]], axis=1).astype(ml_dtypes.bfloat16))
        owc = np.ascontiguousarray(
            o_w[:, c * DQ:(c + 1) * DQ].astype(ml_dtypes.bfloat16))
        in_maps.append({"xT": xT, "wqkv": wqkv, "ow": owc,
                        "cos_t": cos_t, "sin_t": sin_t, "stair": stair})
    return in_maps


def run(inputs: dict, trace: bool = False):
    """Run on the 8 NeuronCores; returns (full_output, BassKernelResults)."""
    nc = build_program()
    in_maps = make_in_maps(**inputs)
    res = run_bass_kernel_spmd(nc, in_maps, core_ids=list(range(NCORES)),
                               trace=trace)
    full = np.concatenate([res.results[c]["out"] for c in range(NCORES)], axis=1)
    return full.reshape(1, S, HIDDEN), res


def kernel(**inputs) -> np.ndarray:
    out, _ = run(inputs)
    return out
